# revision 9
# baseline (speedup 1.0000x reference)
"""Trainium2 Bass kernel: nn_EquivariantCGLayer (GNN message passing).

8 NeuronCores, tgt-sharded: core c owns nodes [c*12500,(c+1)*12500) and all
edges targeting them. Host pre-lays edges into a uniform [128 x T_pad] slot
space (degree-sorted node tiles, exact per-node windows). Device: f[src] via
chunked dma_gather of an 8-node-packed f16 table + 8-way mask-select, f[tgt]
via on-chip scalar broadcast, 48 CG raw components, two-stage tanh on ACT
(scale-grouped), per-slot messages -> DRAM msgbuf -> windowed dma_gather +
masked reduce -> node aggregates -> invariants + MLP (PE) + sigmoid gating.
"""

import math

import numpy as np

P = 128
N_NODES = 100000
NCORES = 8
NSH = N_NODES // NCORES
NTILES = math.ceil(NSH / P)          # 98
NPAD = NTILES * P                    # 12544
SQ3 = float(np.sqrt(3.0))
SQ6 = float(np.sqrt(6.0))
NI = 1024                            # descriptors per dma_gather
GCOLS = NI // P                      # 8
CHUNK = 256                          # slot columns per compute chunk
SUB = 64                             # slot columns per gather sub-chunk

GA, GD, GB, GC, GE = 0, 5, 17, 21, 36
NRAW = 48

_CACHE = {}


def _wrap_idx(logical):
    """int16 logical desc stream -> dma_gather wrapped [128, n/16] layout."""
    lg = logical.reshape(-1, 16)
    w = np.zeros((P, lg.shape[0]), np.int16)
    for g in range(8):
        w[16 * g:16 * g + 16, :] = lg.T
    return w


def _host_layout(edge_index, d, a):
    src = np.asarray(edge_index[0]).astype(np.int64)
    tgt = np.asarray(edge_index[1]).astype(np.int64)
    d = np.asarray(d, dtype=np.float32).reshape(-1)
    a = np.asarray(a, dtype=np.float32)

    cores = []
    for c in range(NCORES):
        sel = (tgt >= c * NSH) & (tgt < (c + 1) * NSH)
        cores.append(dict(src=src[sel], tgt=tgt[sel] - c * NSH,
                          a=a[sel], d=d[sel]))

    for co in cores:
        deg = np.bincount(co["tgt"], minlength=NSH)
        order = np.argsort(-deg, kind="stable")
        co["node_perm"] = np.concatenate([order, np.zeros(NPAD - NSH, np.int64)])
        co["deg_p"] = np.concatenate([deg[order], np.zeros(NPAD - NSH, np.int64)])

    W = np.ones(NTILES, np.int64)
    for co in cores:
        W = np.maximum(W, co["deg_p"].reshape(NTILES, P).max(axis=1))
    c0 = np.concatenate([[0], np.cumsum(W)]).astype(np.int64)
    T_tot = int(c0[-1])
    T_pad = math.ceil(T_tot / CHUNK) * CHUNK
    NG = T_pad // GCOLS

    # msgbuf window rows: record = 8 f16 = 16B, 16 records per 256B row
    R = np.ones(NTILES, np.int64)
    for co in cores:
        deg_pt = co["deg_p"].reshape(NTILES, P)
        for t in range(NTILES):
            st = np.arange(P) * T_pad + int(c0[t])
            en = st + np.maximum(deg_pt[t], 1) - 1
            R[t] = max(R[t], int(((en >> 4) - (st >> 4) + 1).max()))
    cumR = np.concatenate([[0], np.cumsum(R)]).astype(np.int64)
    RT = int(cumR[-1])

    for co in cores:
        a4 = np.zeros((P, T_pad, 4), np.float16)
        dpl = np.zeros((P, T_pad), np.float16)
        srclo8 = np.full((P, T_pad, 8), 64.0, np.float16)
        srcpk = np.zeros((P, T_pad), np.int16)
        eorder = np.argsort(co["tgt"], kind="stable")
        tse = co["tgt"][eorder]
        starts = np.searchsorted(tse, np.arange(NSH))
        ends = np.searchsorted(tse, np.arange(NSH) + 1)
        deg_pt = co["deg_p"].reshape(NTILES, P)
        for t in range(NTILES):
            cs = int(c0[t])
            for p in range(P):
                node = co["node_perm"][t * P + p]
                k = int(deg_pt[t, p])
                if k == 0:
                    continue
                eids = eorder[starts[node]:ends[node]]
                a4[p, cs:cs + k] = co["a"][eids]
                dpl[p, cs:cs + k] = co["d"][eids]
                sv = co["src"][eids]
                srclo8[p, cs:cs + k, :] = (sv & 7)[:, None]
                srcpk[p, cs:cs + k] = (sv >> 3).astype(np.int16)
        co["a4"] = a4.reshape(P, T_pad * 4)
        co["dpl"] = dpl
        co["srclo8"] = srclo8.reshape(P, T_pad * 8)
        co["invc"] = (
            1.0 / (co["deg_p"].reshape(NTILES, P).T.astype(np.float32) + 1e-8)
        ).astype(np.float32)  # [P, NTILES]

        # gather-1 idx stream
        logical1 = np.zeros(NG * NI, np.int16)
        for k in range(NG):
            logical1[k * NI:(k + 1) * NI] = (
                srcpk[:, k * GCOLS:(k + 1) * GCOLS].T.reshape(-1)
            )
        co["idx1"] = _wrap_idx(logical1)

        # gather-2 idx stream + masks
        logical2 = np.zeros(RT * P, np.int16)
        mask2 = np.zeros((P, RT, 16), np.float16)
        for t in range(NTILES):
            cs = int(c0[t])
            st = np.arange(P) * T_pad + cs                     # [P]
            deg = deg_pt[t]
            r0 = st >> 4
            for k in range(int(R[t])):
                col = int(cumR[t]) + k
                logical2[col * P:(col + 1) * P] = (r0 + k).astype(np.int16)
                base = (r0 + k) * 16
                rec = base[:, None] + np.arange(16)[None, :]   # [P,16]
                valid = (rec >= st[:, None]) & (rec < (st + deg)[:, None])
                mask2[:, col, :] = valid.astype(np.float16)
        co["idx2"] = _wrap_idx(logical2)
        # replicate mask over the 8 comps -> [P, RT*128]
        co["mask2"] = np.repeat(
            mask2.reshape(P, RT * 16), 8, axis=1
        ).astype(np.float16)

    meta = dict(W=W, c0=c0, T_pad=T_pad, NG=NG, R=R, cumR=cumR, RT=RT)
    return cores, meta


def _pack_ftab(f):
    nrows = math.ceil(N_NODES / 8)
    ft = np.zeros((nrows * 8, 16), np.float16)
    ft[:N_NODES, :8] = f
    return ft.reshape(nrows, 128)


def _build_nc(meta, w1, w2):
    import concourse.bacc as bacc
    import concourse.mybir as mybir
    from concourse.tile import TileContext
    from concourse.masks import make_identity
    import contextlib

    dt = mybir.dt
    F16, F32 = dt.float16, dt.float32
    AF = mybir.ActivationFunctionType
    ALU = mybir.AluOpType
    T_pad, NG, RT = meta["T_pad"], meta["NG"], meta["RT"]
    W, c0, R, cumR = meta["W"], meta["c0"], meta["R"], meta["cumR"]
    NT = NTILES
    NROWS_FT = math.ceil(N_NODES / 8)
    MROWS = P * T_pad * 8 // 128     # msgbuf 256B rows

    w1 = [float(w1[0]), float(w1[1])]
    w2 = [float(w2[0]), float(w2[1])]

    nc = bacc.Bacc(None, target_bir_lowering=False)
    ftab = nc.dram_tensor("ftab", [NROWS_FT, 128], F16, kind="ExternalInput")
    a4_d = nc.dram_tensor("a4", [P, T_pad * 4], F16, kind="ExternalInput")
    dpl_d = nc.dram_tensor("dpl", [P, T_pad], F16, kind="ExternalInput")
    srclo8_d = nc.dram_tensor("srclo8", [P, T_pad * 8], F16, kind="ExternalInput")
    idx1_d = nc.dram_tensor("idx1", [P, NG * NI // 16], dt.int16, kind="ExternalInput")
    idx2_d = nc.dram_tensor("idx2", [P, RT * P // 16], dt.int16, kind="ExternalInput")
    mask2_d = nc.dram_tensor("mask2", [P, RT * 128], F16, kind="ExternalInput")
    fall_d = nc.dram_tensor("fall", [P, NT * 8], F32, kind="ExternalInput")
    invc_d = nc.dram_tensor("invc", [P, NT], F32, kind="ExternalInput")
    W0_d = nc.dram_tensor("W0", [9, 64], F32, kind="ExternalInput")
    W1_d = nc.dram_tensor("W1", [64, 32], F32, kind="ExternalInput")
    W2_d = nc.dram_tensor("W2", [32, 4], F32, kind="ExternalInput")
    b0_d = nc.dram_tensor("b0", [64, 1], F32, kind="ExternalInput")
    b1_d = nc.dram_tensor("b1", [32, 1], F32, kind="ExternalInput")
    b2_d = nc.dram_tensor("b2", [4, 1], F32, kind="ExternalInput")
    y_d = nc.dram_tensor("y", [P, NT * 8], F32, kind="ExternalOutput")
    msg_d = nc.dram_tensor("msgbuf", [MROWS, 128], F16)

    msg_w = msg_d[:, :].rearrange("r e -> (r e)").rearrange("(p x) -> p x", p=P)

    NCH = T_pad // CHUNK
    with TileContext(nc) as tc:
        with contextlib.ExitStack() as ctx:
            pool = ctx.enter_context(tc.tile_pool(name="sbuf", bufs=2))
            rawp = ctx.enter_context(tc.tile_pool(name="rawp", bufs=1))
            pers = ctx.enter_context(tc.tile_pool(name="pers", bufs=1))
            psum = ctx.enter_context(tc.tile_pool(name="psum", bufs=1, space="PSUM"))

            idx1_t = pers.tile([P, NG * NI // 16], dt.int16, tag="idx1")
            nc.sync.dma_start(idx1_t[:], idx1_d[:])
            idx2_t = pers.tile([P, RT * P // 16], dt.int16, tag="idx2")
            nc.sync.dma_start(idx2_t[:], idx2_d[:])
            fall_t = pers.tile([P, NT * 8], F32, tag="fall")
            nc.sync.dma_start(fall_t[:], fall_d[:])
            fall16 = pers.tile([P, NT * 8], F16, tag="fall16")
            nc.vector.tensor_copy(fall16[:], fall_t[:])
            agg_t = pers.tile([P, NT * 8], F32, tag="agg")

            # ---- ftwin: broadcast f[tgt] along windows ----------------------
            ftwin = pers.tile([P, T_pad * 8], F16, tag="ftwin")
            nc.vector.memset(ftwin[:], 0.0)
            for t in range(NT):
                cs, w = int(c0[t]), int(W[t])
                nc.vector.tensor_copy(
                    ftwin[:, cs * 8:(cs + w) * 8]
                    .rearrange("p (i e) -> p i e", e=8),
                    fall16[:, t * 8:(t + 1) * 8]
                    .rearrange("p (one e) -> p one e", one=1)
                    .to_broadcast([P, w, 8]),
                )

            # ---- edge pipeline ---------------------------------------------
            for ch in range(NCH):
                cs = ch * CHUNK
                fs8 = pool.tile([P, CHUNK * 8], F16, tag="fs8")
                for sub in range(CHUNK // SUB):
                    fswin = pool.tile([P, SUB * P], F16, tag="fswin")
                    for g4 in range(SUB // GCOLS):
                        k = (cs + sub * SUB) // GCOLS + g4
                        nc.gpsimd.dma_gather(
                            out_ap=fswin[:, g4 * GCOLS * P:(g4 + 1) * GCOLS * P]
                            .rearrange("p (i e) -> p i e", e=P),
                            in_ap=ftab[:, :],
                            idxs_ap=idx1_t[:, k * (NI // 16):(k + 1) * (NI // 16)],
                            num_idxs=NI, num_idxs_reg=NI, elem_size=P,
                        )
                    sc0 = cs + sub * SUB
                    slo = pool.tile([P, SUB * 8], F16, tag="slo")
                    nc.sync.dma_start(slo[:], srclo8_d[:, sc0 * 8:(sc0 + SUB) * 8])
                    dst = fs8[:, sub * SUB * 8:(sub + 1) * SUB * 8]
                    for s in range(8):
                        m_s = pool.tile([P, SUB * 8], F16, tag="msk")
                        nc.vector.tensor_scalar(
                            out=m_s[:], in0=slo[:], scalar1=float(s),
                            scalar2=None, op0=ALU.is_equal,
                        )
                        win_s = (fswin[:].rearrange("p (i e) -> p i e", e=P)
                                 [:, :, 16 * s:16 * s + 8])
                        tmp = pool.tile([P, SUB * 8], F16, tag="seltmp")
                        nc.vector.tensor_tensor(
                            out=tmp[:].rearrange("p (i e) -> p i e", e=8),
                            in0=win_s,
                            in1=m_s[:].rearrange("p (i e) -> p i e", e=8),
                            op=ALU.mult,
                        )
                        if s == 0:
                            nc.vector.tensor_copy(dst, tmp[:])
                        else:
                            nc.vector.tensor_tensor(out=dst, in0=dst,
                                                    in1=tmp[:], op=ALU.add)

                a4c = pool.tile([P, CHUNK * 4], F16, tag="a4c")
                nc.sync.dma_start(a4c[:], a4_d[:, cs * 4:(cs + CHUNK) * 4])
                dc = pool.tile([P, CHUNK], F16, tag="dc")
                nc.sync.dma_start(dc[:], dpl_d[:, cs:cs + CHUNK])

                def A4(j):
                    return a4c[:, j::4]

                def FS(c):
                    return fs8[:, c::8]

                def FT(c):
                    return ftwin[:, cs * 8 + c:(cs + CHUNK) * 8:8]

                raw = rawp.tile([P, CHUNK * NRAW], F16, tag="raw")

                def RW(pl):
                    return raw[:, pl * CHUNK:(pl + 1) * CHUNK]

                def mul(o, x, y):
                    nc.vector.tensor_tensor(out=o, in0=x, in1=y, op=ALU.mult)

                def add(o, x, y):
                    nc.vector.tensor_tensor(out=o, in0=x, in1=y, op=ALU.add)

                def sub_(o, x, y):
                    nc.vector.tensor_tensor(out=o, in0=x, in1=y, op=ALU.subtract)

                tmp1 = pool.tile([P, CHUNK], F16, tag="tmp1")
                tmp2 = pool.tile([P, CHUNK], F16, tag="tmp2")
                a0 = A4(0)
                av = [A4(1), A4(2), A4(3)]
                vecs = [(FS(2), FS(3), FS(4)), (FS(5), FS(6), FS(7)),
                        (FT(2), FT(3), FT(4)), (FT(5), FT(6), FT(7))]

                for i, s in enumerate([FS(0), FS(1), FT(0), FT(1), dc[:]]):
                    mul(RW(GA + i), s, a0)
                for vi, v in enumerate(vecs):
                    for j in range(3):
                        mul(RW(GD + vi * 3 + j), v[j], a0)
                for vi, v in enumerate(vecs):
                    mul(tmp1[:], v[0], av[0])
                    mul(tmp2[:], v[1], av[1])
                    add(tmp1[:], tmp1[:], tmp2[:])
                    mul(tmp2[:], v[2], av[2])
                    add(RW(GB + vi), tmp1[:], tmp2[:])
                for i, s in enumerate([FS(0), FS(1), FT(0), FT(1), dc[:]]):
                    for j in range(3):
                        mul(RW(GC + i * 3 + j), s, av[j])
                for vi, v in enumerate(vecs):
                    pl = GE + vi * 3
                    mul(tmp1[:], v[1], av[2]); mul(tmp2[:], v[2], av[1])
                    sub_(RW(pl + 0), tmp1[:], tmp2[:])
                    mul(tmp1[:], v[2], av[0]); mul(tmp2[:], v[0], av[2])
                    sub_(RW(pl + 1), tmp1[:], tmp2[:])
                    mul(tmp1[:], v[0], av[1]); mul(tmp2[:], v[1], av[0])
                    sub_(RW(pl + 2), tmp1[:], tmp2[:])

                t1 = rawp.tile([P, CHUNK * NRAW], F16, tag="t1")
                for (pl0, npl, sc) in [(GA, 5, w1[0]), (GD, 12, w1[0] / SQ3),
                                       (GB, 4, w1[1] / SQ3), (GC, 15, w1[1] / SQ3),
                                       (GE, 12, w1[1] / SQ6)]:
                    nc.scalar.activation(
                        t1[:, pl0 * CHUNK:(pl0 + npl) * CHUNK],
                        raw[:, pl0 * CHUNK:(pl0 + npl) * CHUNK],
                        AF.Tanh, scale=float(sc),
                    )
                t2 = raw  # raw is dead after t1; reuse its buffer
                nc.scalar.activation(t2[:, :17 * CHUNK], t1[:, :17 * CHUNK],
                                     AF.Tanh, scale=w2[0])
                nc.scalar.activation(t2[:, 17 * CHUNK:], t1[:, 17 * CHUNK:],
                                     AF.Tanh, scale=w2[1])

                def T2(pl):
                    return t2[:, pl * CHUNK:(pl + 1) * CHUNK]

                mout = pool.tile([P, CHUNK * 8], F16, tag="mout")
                m0 = tmp1
                add(m0[:], T2(GA + 0), T2(GA + 1))
                for pl in [GA + 2, GA + 3, GA + 4, GB, GB + 1, GB + 2, GB + 3]:
                    add(m0[:], m0[:], T2(pl))
                nc.vector.tensor_copy(mout[:, 0::8], m0[:])
                for j in range(3):
                    mv = tmp2
                    add(mv[:], T2(GC + j), T2(GC + 3 + j))
                    for b in [GC + 6 + j, GC + 9 + j, GC + 12 + j,
                              GD + j, GD + 3 + j, GD + 6 + j, GD + 9 + j,
                              GE + j, GE + 3 + j, GE + 6 + j, GE + 9 + j]:
                        add(mv[:], mv[:], T2(b))
                    nc.vector.tensor_copy(mout[:, 1 + j::8], mv[:])
                nc.vector.tensor_copy(mout[:, 4::8], dc[:])
                nc.vector.memset(mout[:, 5::8], 0.0)
                nc.vector.memset(mout[:, 6::8], 0.0)
                nc.vector.memset(mout[:, 7::8], 0.0)
                nc.sync.dma_start(msg_w[:, cs * 8:(cs + CHUNK) * 8], mout[:])

            # ---- aggregation ----------------------------------------------
            for t in range(NT):
                r0, rt = int(cumR[t]), int(R[t])
                w2b = pool.tile([P, rt * P], F16, tag="w2b")
                done = 0
                while done < rt:
                    ncols = min(GCOLS, rt - done)
                    k0 = r0 + done
                    nc.gpsimd.dma_gather(
                        out_ap=w2b[:, done * P:(done + ncols) * P]
                        .rearrange("p (i e) -> p i e", e=P),
                        in_ap=msg_d[:, :],
                        idxs_ap=idx2_t[:, k0 * 8:(k0 + ncols) * 8],
                        num_idxs=ncols * P, num_idxs_reg=ncols * P, elem_size=P,
                    )
                    done += ncols
                m2 = pool.tile([P, rt * 128], F16, tag="m2")
                nc.sync.dma_start(m2[:], mask2_d[:, r0 * 128:(r0 + rt) * 128])
                nc.vector.tensor_tensor(out=w2b[:], in0=w2b[:], in1=m2[:],
                                        op=ALU.mult)
                nc.vector.tensor_reduce(
                    out=agg_t[:, t * 8:(t + 1) * 8],
                    in_=w2b[:].rearrange("p (x c) -> p c x", c=8),
                    axis=mybir.AxisListType.X, op=ALU.add,
                )

            # ---- node stage ------------------------------------------------
            invc_t = pers.tile([P, NT], F32, tag="invc")
            nc.sync.dma_start(invc_t[:], invc_d[:])
            ident = pers.tile([P, P], F32, tag="ident")
            make_identity(nc, ident[:])
            w0t = pers.tile([9, 64], F32, tag="w0")
            nc.sync.dma_start(w0t[:], W0_d[:])
            w1t = pers.tile([64, 32], F32, tag="w1")
            nc.sync.dma_start(w1t[:], W1_d[:])
            w2t = pers.tile([32, 4], F32, tag="w2")
            nc.sync.dma_start(w2t[:], W2_d[:])
            b0t = pers.tile([64, 1], F32, tag="b0")
            nc.sync.dma_start(b0t[:], b0_d[:])
            b1t = pers.tile([32, 1], F32, tag="b1")
            nc.sync.dma_start(b1t[:], b1_d[:])
            b2t = pers.tile([4, 1], F32, tag="b2")
            nc.sync.dma_start(b2t[:], b2_d[:])

            psi = pers.tile([P, NT * 9], F32, tag="psi")
            tm1 = pers.tile([P, NT], F32, tag="tm1")
            tm2 = pers.tile([P, NT], F32, tag="tm2")

            def FA(c):
                return fall_t[:, c::8]

            def AG(c):
                return agg_t[:, c::8]

            def vmul(o, x, y):
                nc.vector.tensor_tensor(out=o, in0=x, in1=y, op=ALU.mult)

            def vadd(o, x, y):
                nc.vector.tensor_tensor(out=o, in0=x, in1=y, op=ALU.add)

            # psi0..1 = f0, f1 ; psi2 = |f[2:5]| ; psi3 = |f[5:8]|
            nc.vector.tensor_copy(psi[:, 0::9], FA(0))
            nc.vector.tensor_copy(psi[:, 1::9], FA(1))
            for (k, base) in [(2, 2), (3, 5)]:
                vmul(tm1[:], FA(base), FA(base))
                vmul(tm2[:], FA(base + 1), FA(base + 1))
                vadd(tm1[:], tm1[:], tm2[:])
                vmul(tm2[:], FA(base + 2), FA(base + 2))
                vadd(tm1[:], tm1[:], tm2[:])
                nc.scalar.activation(psi[:, k::9], tm1[:], AF.Sqrt)
            # psi4,5 = m0 ; psi6,7 = |mv|
            nc.vector.tensor_copy(psi[:, 4::9], AG(0))
            nc.vector.tensor_copy(psi[:, 5::9], AG(0))
            vmul(tm1[:], AG(1), AG(1))
            vmul(tm2[:], AG(2), AG(2))
            vadd(tm1[:], tm1[:], tm2[:])
            vmul(tm2[:], AG(3), AG(3))
            vadd(tm1[:], tm1[:], tm2[:])
            nc.scalar.activation(psi[:, 6::9], tm1[:], AF.Sqrt)
            nc.vector.tensor_copy(psi[:, 7::9], psi[:, 6::9])
            # psi8 = avg_d
            vmul(psi[:, 8::9], AG(4), invc_t[:])

            gm = pers.tile([P, NT * 4], F32, tag="gm")
            NCHN = 4                      # tiles per node chunk (512 nodes)
            for q in range(NT // NCHN + (1 if NT % NCHN else 0)):
                tlo = q * NCHN
                thi = min(tlo + NCHN, NT)
                nt_q = thi - tlo
                psiT = pool.tile([9, nt_q * P], F32, tag="psiT")
                for ti in range(nt_q):
                    pst = psum.tile([9, P], F32, tag="pst", space="PSUM")
                    nc.tensor.transpose(
                        out=pst[:],
                        in_=psi[:, (tlo + ti) * 9:(tlo + ti + 1) * 9],
                        identity=ident[:],
                    )
                    nc.vector.tensor_copy(psiT[:, ti * P:(ti + 1) * P], pst[:])
                x1p = psum.tile([64, nt_q * P], F32, tag="x1p", space="PSUM")
                nc.tensor.matmul(x1p[:], lhsT=w0t[:], rhs=psiT[:], start=True,
                                 stop=True)
                x1s = pool.tile([64, nt_q * P], F32, tag="x1s")
                nc.scalar.activation(x1s[:], x1p[:], AF.Relu, bias=b0t[:, 0:1])
                x2p = psum.tile([32, nt_q * P], F32, tag="x2p", space="PSUM")
                nc.tensor.matmul(x2p[:], lhsT=w1t[:], rhs=x1s[:], start=True,
                                 stop=True)
                x2s = pool.tile([32, nt_q * P], F32, tag="x2s")
                nc.scalar.activation(x2s[:], x2p[:], AF.Relu, bias=b1t[:, 0:1])
                gp = psum.tile([4, nt_q * P], F32, tag="gp", space="PSUM")
                nc.tensor.matmul(gp[:], lhsT=w2t[:], rhs=x2s[:], start=True,
                                 stop=True)
                gs = pool.tile([4, nt_q * P], F32, tag="gs")
                nc.scalar.activation(gs[:], gp[:], AF.Sigmoid, bias=b2t[:, 0:1])
                for ti in range(nt_q):
                    gb = psum.tile([P, 4], F32, tag="gb", space="PSUM")
                    nc.tensor.transpose(
                        out=gb[:], in_=gs[:, ti * P:(ti + 1) * P],
                        identity=ident[:4, :4],
                    )
                    nc.vector.tensor_copy(
                        gm[:, (tlo + ti) * 4:(tlo + ti + 1) * 4], gb[:]
                    )

            y_t = pers.tile([P, NT * 8], F32, tag="y")
            gidx = [0, 1, 2, 2, 2, 3, 3, 3]
            aidx = [0, 0, 1, 2, 3, 1, 2, 3]
            for c in range(8):
                vmul(tm1[:], gm[:, gidx[c]::4], AG(aidx[c]))
                vadd(y_t[:, c::8], FA(c), tm1[:])
            nc.sync.dma_start(y_d[:], y_t[:])

    nc.finalize()
    return nc


def kernel(edge_index, f, d, a, w1, w2, W0, b0, W1, b1, W2, b2):
    from concourse.bass_utils import run_bass_kernel_spmd

    f = np.asarray(f, dtype=np.float32)
    w1 = np.asarray(w1, dtype=np.float32)
    w2 = np.asarray(w2, dtype=np.float32)

    cores, meta = _host_layout(edge_index, d, a)
    ftab = _pack_ftab(f)

    key = (meta["T_pad"], meta["RT"], tuple(meta["W"]), tuple(meta["R"]),
           float(w1[0]), float(w1[1]), float(w2[0]), float(w2[1]))
    if key not in _CACHE:
        _CACHE[key] = _build_nc(meta, w1, w2)
    nc = _CACHE[key]

    in_maps = []
    for c, co in enumerate(cores):
        fall = np.zeros((P, NTILES * 8), np.float32)
        node_ids = co["node_perm"].reshape(NTILES, P)
        for t in range(NTILES):
            fall[:, t * 8:(t + 1) * 8] = f[c * NSH + node_ids[t]]
        in_maps.append({
            "ftab": ftab,
            "a4": co["a4"], "dpl": co["dpl"], "srclo8": co["srclo8"],
            "idx1": co["idx1"], "idx2": co["idx2"], "mask2": co["mask2"],
            "fall": fall, "invc": co["invc"],
            "W0": np.asarray(W0, np.float32),
            "W1": np.asarray(W1, np.float32),
            "W2": np.asarray(W2, np.float32),
            "b0": np.asarray(b0, np.float32).reshape(64, 1),
            "b1": np.asarray(b1, np.float32).reshape(32, 1),
            "b2": np.asarray(b2, np.float32).reshape(4, 1),
        })

    res = run_bass_kernel_spmd(nc, in_maps, core_ids=list(range(NCORES)))

    out = np.zeros((N_NODES, 8), np.float32)
    for c, co in enumerate(cores):
        y = res.results[c]["y"].reshape(P, NTILES, 8)
        node_ids = co["node_perm"].reshape(NTILES, P)
        for t in range(NTILES):
            valid = (t * P + np.arange(P)) < NSH
            out[c * NSH + node_ids[t][valid]] = y[valid, t]
    return out


# revision 11
# speedup vs baseline: 1.0190x; 1.0190x over previous
"""Trainium2 Bass kernel: nn_EquivariantCGLayer (GNN message passing).

8 NeuronCores, tgt-sharded: core c owns nodes [c*12500,(c+1)*12500) and all
edges targeting them. Host pre-lays edges into a uniform [128 x T_pad] slot
space (degree-sorted node tiles, exact per-node windows). Device: f[src] via
chunked dma_gather of an 8-node-packed f16 table + 8-way mask-select, f[tgt]
via on-chip scalar broadcast, 48 CG raw components, two-stage tanh on ACT
(scale-grouped), per-slot messages -> DRAM msgbuf -> windowed dma_gather +
masked reduce -> node aggregates -> invariants + MLP (PE) + sigmoid gating.
"""

import math

import numpy as np

P = 128
N_NODES = 100000
NCORES = 8
NSH = N_NODES // NCORES
NTILES = math.ceil(NSH / P)          # 98
NPAD = NTILES * P                    # 12544
SQ3 = float(np.sqrt(3.0))
SQ6 = float(np.sqrt(6.0))
NI = 1024                            # descriptors per dma_gather
GCOLS = NI // P                      # 8
CHUNK = 256                          # slot columns per compute chunk
SUB = 64                             # slot columns per gather sub-chunk

GA, GD, GB, GC, GE = 0, 5, 17, 21, 36
NRAW = 48

_CACHE = {}


def _wrap_idx(logical):
    """int16 logical desc stream -> dma_gather wrapped [128, n/16] layout."""
    lg = logical.reshape(-1, 16)
    w = np.zeros((P, lg.shape[0]), np.int16)
    for g in range(8):
        w[16 * g:16 * g + 16, :] = lg.T
    return w


def _host_layout(edge_index, d, a):
    src = np.asarray(edge_index[0]).astype(np.int64)
    tgt = np.asarray(edge_index[1]).astype(np.int64)
    d = np.asarray(d, dtype=np.float32).reshape(-1)
    a = np.asarray(a, dtype=np.float32)

    cores = []
    for c in range(NCORES):
        sel = (tgt >= c * NSH) & (tgt < (c + 1) * NSH)
        cores.append(dict(src=src[sel], tgt=tgt[sel] - c * NSH,
                          a=a[sel], d=d[sel]))

    for co in cores:
        deg = np.bincount(co["tgt"], minlength=NSH)
        order = np.argsort(-deg, kind="stable")
        co["node_perm"] = np.concatenate([order, np.zeros(NPAD - NSH, np.int64)])
        co["deg_p"] = np.concatenate([deg[order], np.zeros(NPAD - NSH, np.int64)])

    W = np.ones(NTILES, np.int64)
    for co in cores:
        W = np.maximum(W, co["deg_p"].reshape(NTILES, P).max(axis=1))
    c0 = np.concatenate([[0], np.cumsum(W)]).astype(np.int64)
    T_tot = int(c0[-1])
    T_pad = math.ceil(T_tot / CHUNK) * CHUNK
    NG = T_pad // GCOLS

    # msgbuf window rows: record = 8 f16 = 16B, 16 records per 256B row
    R = np.ones(NTILES, np.int64)
    for co in cores:
        deg_pt = co["deg_p"].reshape(NTILES, P)
        for t in range(NTILES):
            st = np.arange(P) * T_pad + int(c0[t])
            en = st + np.maximum(deg_pt[t], 1) - 1
            R[t] = max(R[t], int(((en >> 4) - (st >> 4) + 1).max()))
    cumR = np.concatenate([[0], np.cumsum(R)]).astype(np.int64)
    RT = int(cumR[-1])

    for co in cores:
        a4 = np.zeros((P, T_pad, 4), np.float16)
        dpl = np.zeros((P, T_pad), np.float16)
        srclo8 = np.full((P, T_pad, 8), 64.0, np.float16)
        srcpk = np.zeros((P, T_pad), np.int16)
        eorder = np.argsort(co["tgt"], kind="stable")
        tse = co["tgt"][eorder]
        starts = np.searchsorted(tse, np.arange(NSH))
        ends = np.searchsorted(tse, np.arange(NSH) + 1)
        deg_pt = co["deg_p"].reshape(NTILES, P)
        for t in range(NTILES):
            cs = int(c0[t])
            for p in range(P):
                node = co["node_perm"][t * P + p]
                k = int(deg_pt[t, p])
                if k == 0:
                    continue
                eids = eorder[starts[node]:ends[node]]
                a4[p, cs:cs + k] = co["a"][eids]
                dpl[p, cs:cs + k] = co["d"][eids]
                sv = co["src"][eids]
                srclo8[p, cs:cs + k, :] = (sv & 7)[:, None]
                srcpk[p, cs:cs + k] = (sv >> 3).astype(np.int16)
        co["a4"] = a4.reshape(P, T_pad * 4)
        co["dpl"] = dpl
        co["srclo8"] = srclo8.reshape(P, T_pad * 8)
        co["invc"] = (
            1.0 / (co["deg_p"].reshape(NTILES, P).T.astype(np.float32) + 1e-8)
        ).astype(np.float32)  # [P, NTILES]

        # gather-1 idx stream
        logical1 = np.zeros(NG * NI, np.int16)
        for k in range(NG):
            logical1[k * NI:(k + 1) * NI] = (
                srcpk[:, k * GCOLS:(k + 1) * GCOLS].T.reshape(-1)
            )
        co["idx1"] = _wrap_idx(logical1)

        # gather-2 idx stream + masks
        logical2 = np.zeros(RT * P, np.int16)
        mask2 = np.zeros((P, RT, 16), np.float16)
        for t in range(NTILES):
            cs = int(c0[t])
            st = np.arange(P) * T_pad + cs                     # [P]
            deg = deg_pt[t]
            r0 = st >> 4
            for k in range(int(R[t])):
                col = int(cumR[t]) + k
                logical2[col * P:(col + 1) * P] = (r0 + k).astype(np.int16)
                base = (r0 + k) * 16
                rec = base[:, None] + np.arange(16)[None, :]   # [P,16]
                valid = (rec >= st[:, None]) & (rec < (st + deg)[:, None])
                mask2[:, col, :] = valid.astype(np.float16)
        co["idx2"] = _wrap_idx(logical2)
        # replicate mask over the 8 comps -> [P, RT*128]
        co["mask2"] = np.repeat(
            mask2.reshape(P, RT * 16), 8, axis=1
        ).astype(np.float16)

    meta = dict(W=W, c0=c0, T_pad=T_pad, NG=NG, R=R, cumR=cumR, RT=RT)
    return cores, meta


def _pack_ftab(f):
    nrows = math.ceil(N_NODES / 8)
    ft = np.zeros((nrows * 8, 16), np.float16)
    ft[:N_NODES, :8] = f
    return ft.reshape(nrows, 128)


def _build_nc(meta, w1, w2):
    import concourse.bacc as bacc
    import concourse.mybir as mybir
    from concourse.tile import TileContext
    from concourse.masks import make_identity
    import contextlib

    dt = mybir.dt
    F16, F32 = dt.float16, dt.float32
    AF = mybir.ActivationFunctionType
    ALU = mybir.AluOpType
    T_pad, NG, RT = meta["T_pad"], meta["NG"], meta["RT"]
    W, c0, R, cumR = meta["W"], meta["c0"], meta["R"], meta["cumR"]
    NT = NTILES
    NROWS_FT = math.ceil(N_NODES / 8)
    MROWS = P * T_pad * 8 // 128     # msgbuf 256B rows

    w1 = [float(w1[0]), float(w1[1])]
    w2 = [float(w2[0]), float(w2[1])]

    nc = bacc.Bacc(None, target_bir_lowering=False)
    ftab = nc.dram_tensor("ftab", [NROWS_FT, 128], F16, kind="ExternalInput")
    a4_d = nc.dram_tensor("a4", [P, T_pad * 4], F16, kind="ExternalInput")
    dpl_d = nc.dram_tensor("dpl", [P, T_pad], F16, kind="ExternalInput")
    srclo8_d = nc.dram_tensor("srclo8", [P, T_pad * 8], F16, kind="ExternalInput")
    idx1_d = nc.dram_tensor("idx1", [P, NG * NI // 16], dt.int16, kind="ExternalInput")
    idx2_d = nc.dram_tensor("idx2", [P, RT * P // 16], dt.int16, kind="ExternalInput")
    mask2_d = nc.dram_tensor("mask2", [P, RT * 128], F16, kind="ExternalInput")
    fall_d = nc.dram_tensor("fall", [P, NT * 8], F32, kind="ExternalInput")
    invc_d = nc.dram_tensor("invc", [P, NT], F32, kind="ExternalInput")
    W0_d = nc.dram_tensor("W0", [9, 64], F32, kind="ExternalInput")
    W1_d = nc.dram_tensor("W1", [64, 32], F32, kind="ExternalInput")
    W2_d = nc.dram_tensor("W2", [32, 4], F32, kind="ExternalInput")
    b0_d = nc.dram_tensor("b0", [64, 1], F32, kind="ExternalInput")
    b1_d = nc.dram_tensor("b1", [32, 1], F32, kind="ExternalInput")
    b2_d = nc.dram_tensor("b2", [4, 1], F32, kind="ExternalInput")
    y_d = nc.dram_tensor("y", [P, NT * 8], F32, kind="ExternalOutput")
    msg_d = nc.dram_tensor("msgbuf", [MROWS, 128], F16)

    msg_w = msg_d[:, :].rearrange("r e -> (r e)").rearrange("(p x) -> p x", p=P)

    NCH = T_pad // CHUNK
    with TileContext(nc) as tc:
        with contextlib.ExitStack() as ctx:
            pool = ctx.enter_context(tc.tile_pool(name="sbuf", bufs=2))
            rawp = ctx.enter_context(tc.tile_pool(name="rawp", bufs=1))
            pers = ctx.enter_context(tc.tile_pool(name="pers", bufs=1))
            psum = ctx.enter_context(tc.tile_pool(name="psum", bufs=1, space="PSUM"))

            idx2_t = pers.tile([P, RT * P // 16], dt.int16, tag="idx2")
            nc.sync.dma_start(idx2_t[:], idx2_d[:])
            fall_t = pers.tile([P, NT * 8], F32, tag="fall")
            nc.sync.dma_start(fall_t[:], fall_d[:])
            fall16 = pers.tile([P, NT * 8], F16, tag="fall16")
            nc.vector.tensor_copy(fall16[:], fall_t[:])
            agg_t = pers.tile([P, NT * 8], F32, tag="agg")

            # ---- ftwin: broadcast f[tgt] along windows ----------------------
            ftwin = pers.tile([P, T_pad * 8], F16, tag="ftwin")
            nc.vector.memset(ftwin[:], 0.0)
            for t in range(NT):
                cs, w = int(c0[t]), int(W[t])
                nc.vector.tensor_copy(
                    ftwin[:, cs * 8:(cs + w) * 8]
                    .rearrange("p (i e) -> p i e", e=8),
                    fall16[:, t * 8:(t + 1) * 8]
                    .rearrange("p (one e) -> p one e", one=1)
                    .to_broadcast([P, w, 8]),
                )

            # ---- edge pipeline ---------------------------------------------
            for ch in range(NCH):
                cs = ch * CHUNK
                nch_i = CHUNK // GCOLS
                idx1_t = pool.tile([P, nch_i * (NI // 16)], dt.int16, tag="idx1c")
                nc.sync.dma_start(
                    idx1_t[:],
                    idx1_d[:, ch * nch_i * (NI // 16):(ch + 1) * nch_i * (NI // 16)],
                )
                fs8 = pool.tile([P, CHUNK * 8], F16, tag="fs8")
                for sub in range(CHUNK // SUB):
                    fswin = pool.tile([P, SUB * P], F16, tag="fswin")
                    for g4 in range(SUB // GCOLS):
                        kk = (sub * SUB) // GCOLS + g4
                        nc.gpsimd.dma_gather(
                            out_ap=fswin[:, g4 * GCOLS * P:(g4 + 1) * GCOLS * P]
                            .rearrange("p (i e) -> p i e", e=P),
                            in_ap=ftab[:, :],
                            idxs_ap=idx1_t[:, kk * (NI // 16):(kk + 1) * (NI // 16)],
                            num_idxs=NI, num_idxs_reg=NI, elem_size=P,
                        )
                    sc0 = cs + sub * SUB
                    slo = pool.tile([P, SUB * 8], F16, tag="slo")
                    nc.sync.dma_start(slo[:], srclo8_d[:, sc0 * 8:(sc0 + SUB) * 8])
                    dst = fs8[:, sub * SUB * 8:(sub + 1) * SUB * 8]
                    for s in range(8):
                        m_s = pool.tile([P, SUB * 8], F16, tag="msk")
                        nc.vector.tensor_scalar(
                            out=m_s[:], in0=slo[:], scalar1=float(s),
                            scalar2=None, op0=ALU.is_equal,
                        )
                        win_s = (fswin[:].rearrange("p (i e) -> p i e", e=P)
                                 [:, :, 16 * s:16 * s + 8])
                        tmp = pool.tile([P, SUB * 8], F16, tag="seltmp")
                        nc.vector.tensor_tensor(
                            out=tmp[:].rearrange("p (i e) -> p i e", e=8),
                            in0=win_s,
                            in1=m_s[:].rearrange("p (i e) -> p i e", e=8),
                            op=ALU.mult,
                        )
                        if s == 0:
                            nc.vector.tensor_copy(dst, tmp[:])
                        else:
                            nc.vector.tensor_tensor(out=dst, in0=dst,
                                                    in1=tmp[:], op=ALU.add)

                a4c = pool.tile([P, CHUNK * 4], F16, tag="a4c")
                nc.sync.dma_start(a4c[:], a4_d[:, cs * 4:(cs + CHUNK) * 4])
                dc = pool.tile([P, CHUNK], F16, tag="dc")
                nc.sync.dma_start(dc[:], dpl_d[:, cs:cs + CHUNK])

                def A4(j):
                    return a4c[:, j::4]

                def FS(c):
                    return fs8[:, c::8]

                def FT(c):
                    return ftwin[:, cs * 8 + c:(cs + CHUNK) * 8:8]

                raw = rawp.tile([P, CHUNK * NRAW], F16, tag="raw")

                def RW(pl):
                    return raw[:, pl * CHUNK:(pl + 1) * CHUNK]

                def mul(o, x, y):
                    nc.vector.tensor_tensor(out=o, in0=x, in1=y, op=ALU.mult)

                def add(o, x, y):
                    nc.vector.tensor_tensor(out=o, in0=x, in1=y, op=ALU.add)

                def sub_(o, x, y):
                    nc.vector.tensor_tensor(out=o, in0=x, in1=y, op=ALU.subtract)

                tmp1 = pool.tile([P, CHUNK], F16, tag="tmp1")
                tmp2 = pool.tile([P, CHUNK], F16, tag="tmp2")
                a0 = A4(0)
                av = [A4(1), A4(2), A4(3)]
                vecs = [(FS(2), FS(3), FS(4)), (FS(5), FS(6), FS(7)),
                        (FT(2), FT(3), FT(4)), (FT(5), FT(6), FT(7))]

                for i, s in enumerate([FS(0), FS(1), FT(0), FT(1), dc[:]]):
                    mul(RW(GA + i), s, a0)
                for vi, v in enumerate(vecs):
                    for j in range(3):
                        mul(RW(GD + vi * 3 + j), v[j], a0)
                for vi, v in enumerate(vecs):
                    mul(tmp1[:], v[0], av[0])
                    mul(tmp2[:], v[1], av[1])
                    add(tmp1[:], tmp1[:], tmp2[:])
                    mul(tmp2[:], v[2], av[2])
                    add(RW(GB + vi), tmp1[:], tmp2[:])
                for i, s in enumerate([FS(0), FS(1), FT(0), FT(1), dc[:]]):
                    for j in range(3):
                        mul(RW(GC + i * 3 + j), s, av[j])
                for vi, v in enumerate(vecs):
                    pl = GE + vi * 3
                    mul(tmp1[:], v[1], av[2]); mul(tmp2[:], v[2], av[1])
                    sub_(RW(pl + 0), tmp1[:], tmp2[:])
                    mul(tmp1[:], v[2], av[0]); mul(tmp2[:], v[0], av[2])
                    sub_(RW(pl + 1), tmp1[:], tmp2[:])
                    mul(tmp1[:], v[0], av[1]); mul(tmp2[:], v[1], av[0])
                    sub_(RW(pl + 2), tmp1[:], tmp2[:])

                t1 = rawp.tile([P, CHUNK * NRAW], F16, tag="t1")
                for (pl0, npl, sc) in [(GA, 5, w1[0]), (GD, 12, w1[0] / SQ3),
                                       (GB, 4, w1[1] / SQ3), (GC, 15, w1[1] / SQ3),
                                       (GE, 12, w1[1] / SQ6)]:
                    nc.scalar.activation(
                        t1[:, pl0 * CHUNK:(pl0 + npl) * CHUNK],
                        raw[:, pl0 * CHUNK:(pl0 + npl) * CHUNK],
                        AF.Tanh, scale=float(sc),
                    )
                t2 = raw  # raw is dead after t1; reuse its buffer
                nc.scalar.activation(t2[:, :17 * CHUNK], t1[:, :17 * CHUNK],
                                     AF.Tanh, scale=w2[0])
                nc.scalar.activation(t2[:, 17 * CHUNK:], t1[:, 17 * CHUNK:],
                                     AF.Tanh, scale=w2[1])

                def T2(pl):
                    return t2[:, pl * CHUNK:(pl + 1) * CHUNK]

                mout = pool.tile([P, CHUNK * 8], F16, tag="mout")
                m0 = tmp1
                add(m0[:], T2(GA + 0), T2(GA + 1))
                for pl in [GA + 2, GA + 3, GA + 4, GB, GB + 1, GB + 2, GB + 3]:
                    add(m0[:], m0[:], T2(pl))
                nc.vector.tensor_copy(mout[:, 0::8], m0[:])
                for j in range(3):
                    mv = tmp2
                    add(mv[:], T2(GC + j), T2(GC + 3 + j))
                    for b in [GC + 6 + j, GC + 9 + j, GC + 12 + j,
                              GD + j, GD + 3 + j, GD + 6 + j, GD + 9 + j,
                              GE + j, GE + 3 + j, GE + 6 + j, GE + 9 + j]:
                        add(mv[:], mv[:], T2(b))
                    nc.vector.tensor_copy(mout[:, 1 + j::8], mv[:])
                nc.vector.tensor_copy(mout[:, 4::8], dc[:])
                nc.vector.memset(mout[:, 5::8], 0.0)
                nc.vector.memset(mout[:, 6::8], 0.0)
                nc.vector.memset(mout[:, 7::8], 0.0)
                nc.sync.dma_start(msg_w[:, cs * 8:(cs + CHUNK) * 8], mout[:])

            # ---- aggregation (gathers batched to full instructions) --------
            groups = []
            cur, cols = [], 0
            for t in range(NT):
                rt = int(R[t])
                if cols + rt > GCOLS and cur:
                    groups.append((cur, cols))
                    cur, cols = [], 0
                cur.append(t)
                cols += rt
            if cur:
                groups.append((cur, cols))
            for tiles, cols in groups:
                r0 = int(cumR[tiles[0]])
                w2b = pool.tile([P, cols * P], F16, tag="w2b")
                nc.gpsimd.dma_gather(
                    out_ap=w2b[:].rearrange("p (i e) -> p i e", e=P),
                    in_ap=msg_d[:, :],
                    idxs_ap=idx2_t[:, r0 * 8:(r0 + cols) * 8],
                    num_idxs=cols * P, num_idxs_reg=cols * P, elem_size=P,
                )
                m2 = pool.tile([P, cols * 128], F16, tag="m2")
                nc.sync.dma_start(m2[:], mask2_d[:, r0 * 128:(r0 + cols) * 128])
                nc.vector.tensor_tensor(out=w2b[:], in0=w2b[:], in1=m2[:],
                                        op=ALU.mult)
                off = 0
                for t in tiles:
                    rt = int(R[t])
                    nc.vector.tensor_reduce(
                        out=agg_t[:, t * 8:(t + 1) * 8],
                        in_=w2b[:, off * P:(off + rt) * P]
                        .rearrange("p (x c) -> p c x", c=8),
                        axis=mybir.AxisListType.X, op=ALU.add,
                    )
                    off += rt

            # ---- node stage ------------------------------------------------
            invc_t = pers.tile([P, NT], F32, tag="invc")
            nc.sync.dma_start(invc_t[:], invc_d[:])
            ident = pers.tile([P, P], F32, tag="ident")
            make_identity(nc, ident[:])
            w0t = pers.tile([9, 64], F32, tag="w0")
            nc.sync.dma_start(w0t[:], W0_d[:])
            w1t = pers.tile([64, 32], F32, tag="w1")
            nc.sync.dma_start(w1t[:], W1_d[:])
            w2t = pers.tile([32, 4], F32, tag="w2")
            nc.sync.dma_start(w2t[:], W2_d[:])
            b0t = pers.tile([64, 1], F32, tag="b0")
            nc.sync.dma_start(b0t[:], b0_d[:])
            b1t = pers.tile([32, 1], F32, tag="b1")
            nc.sync.dma_start(b1t[:], b1_d[:])
            b2t = pers.tile([4, 1], F32, tag="b2")
            nc.sync.dma_start(b2t[:], b2_d[:])

            psi = pers.tile([P, NT * 9], F32, tag="psi")
            tm1 = pers.tile([P, NT], F32, tag="tm1")
            tm2 = pers.tile([P, NT], F32, tag="tm2")

            def FA(c):
                return fall_t[:, c::8]

            def AG(c):
                return agg_t[:, c::8]

            def vmul(o, x, y):
                nc.vector.tensor_tensor(out=o, in0=x, in1=y, op=ALU.mult)

            def vadd(o, x, y):
                nc.vector.tensor_tensor(out=o, in0=x, in1=y, op=ALU.add)

            # psi0..1 = f0, f1 ; psi2 = |f[2:5]| ; psi3 = |f[5:8]|
            nc.vector.tensor_copy(psi[:, 0::9], FA(0))
            nc.vector.tensor_copy(psi[:, 1::9], FA(1))
            for (k, base) in [(2, 2), (3, 5)]:
                vmul(tm1[:], FA(base), FA(base))
                vmul(tm2[:], FA(base + 1), FA(base + 1))
                vadd(tm1[:], tm1[:], tm2[:])
                vmul(tm2[:], FA(base + 2), FA(base + 2))
                vadd(tm1[:], tm1[:], tm2[:])
                nc.scalar.activation(psi[:, k::9], tm1[:], AF.Sqrt)
            # psi4,5 = m0 ; psi6,7 = |mv|
            nc.vector.tensor_copy(psi[:, 4::9], AG(0))
            nc.vector.tensor_copy(psi[:, 5::9], AG(0))
            vmul(tm1[:], AG(1), AG(1))
            vmul(tm2[:], AG(2), AG(2))
            vadd(tm1[:], tm1[:], tm2[:])
            vmul(tm2[:], AG(3), AG(3))
            vadd(tm1[:], tm1[:], tm2[:])
            nc.scalar.activation(psi[:, 6::9], tm1[:], AF.Sqrt)
            nc.vector.tensor_copy(psi[:, 7::9], psi[:, 6::9])
            # psi8 = avg_d
            vmul(psi[:, 8::9], AG(4), invc_t[:])

            gm = pers.tile([P, NT * 4], F32, tag="gm")
            NCHN = 4                      # tiles per node chunk (512 nodes)
            for q in range(NT // NCHN + (1 if NT % NCHN else 0)):
                tlo = q * NCHN
                thi = min(tlo + NCHN, NT)
                nt_q = thi - tlo
                psiT = pool.tile([9, nt_q * P], F32, tag="psiT")
                for ti in range(nt_q):
                    pst = psum.tile([9, P], F32, tag="pst", space="PSUM")
                    nc.tensor.transpose(
                        out=pst[:],
                        in_=psi[:, (tlo + ti) * 9:(tlo + ti + 1) * 9],
                        identity=ident[:],
                    )
                    nc.vector.tensor_copy(psiT[:, ti * P:(ti + 1) * P], pst[:])
                x1p = psum.tile([64, nt_q * P], F32, tag="x1p", space="PSUM")
                nc.tensor.matmul(x1p[:], lhsT=w0t[:], rhs=psiT[:], start=True,
                                 stop=True)
                x1s = pool.tile([64, nt_q * P], F32, tag="x1s")
                nc.scalar.activation(x1s[:], x1p[:], AF.Relu, bias=b0t[:, 0:1])
                x2p = psum.tile([32, nt_q * P], F32, tag="x2p", space="PSUM")
                nc.tensor.matmul(x2p[:], lhsT=w1t[:], rhs=x1s[:], start=True,
                                 stop=True)
                x2s = pool.tile([32, nt_q * P], F32, tag="x2s")
                nc.scalar.activation(x2s[:], x2p[:], AF.Relu, bias=b1t[:, 0:1])
                gp = psum.tile([4, nt_q * P], F32, tag="gp", space="PSUM")
                nc.tensor.matmul(gp[:], lhsT=w2t[:], rhs=x2s[:], start=True,
                                 stop=True)
                gs = pool.tile([4, nt_q * P], F32, tag="gs")
                nc.scalar.activation(gs[:], gp[:], AF.Sigmoid, bias=b2t[:, 0:1])
                for ti in range(nt_q):
                    gb = psum.tile([P, 4], F32, tag="gb", space="PSUM")
                    nc.tensor.transpose(
                        out=gb[:], in_=gs[:, ti * P:(ti + 1) * P],
                        identity=ident[:4, :4],
                    )
                    nc.vector.tensor_copy(
                        gm[:, (tlo + ti) * 4:(tlo + ti + 1) * 4], gb[:]
                    )

            y_t = pers.tile([P, NT * 8], F32, tag="y")
            gidx = [0, 1, 2, 2, 2, 3, 3, 3]
            aidx = [0, 0, 1, 2, 3, 1, 2, 3]
            for c in range(8):
                vmul(tm1[:], gm[:, gidx[c]::4], AG(aidx[c]))
                vadd(y_t[:, c::8], FA(c), tm1[:])
            nc.sync.dma_start(y_d[:], y_t[:])

    nc.finalize()
    return nc


def kernel(edge_index, f, d, a, w1, w2, W0, b0, W1, b1, W2, b2):
    from concourse.bass_utils import run_bass_kernel_spmd

    f = np.asarray(f, dtype=np.float32)
    w1 = np.asarray(w1, dtype=np.float32)
    w2 = np.asarray(w2, dtype=np.float32)

    cores, meta = _host_layout(edge_index, d, a)
    ftab = _pack_ftab(f)

    key = (meta["T_pad"], meta["RT"], tuple(meta["W"]), tuple(meta["R"]),
           float(w1[0]), float(w1[1]), float(w2[0]), float(w2[1]))
    if key not in _CACHE:
        _CACHE[key] = _build_nc(meta, w1, w2)
    nc = _CACHE[key]

    in_maps = []
    for c, co in enumerate(cores):
        fall = np.zeros((P, NTILES * 8), np.float32)
        node_ids = co["node_perm"].reshape(NTILES, P)
        for t in range(NTILES):
            fall[:, t * 8:(t + 1) * 8] = f[c * NSH + node_ids[t]]
        in_maps.append({
            "ftab": ftab,
            "a4": co["a4"], "dpl": co["dpl"], "srclo8": co["srclo8"],
            "idx1": co["idx1"], "idx2": co["idx2"], "mask2": co["mask2"],
            "fall": fall, "invc": co["invc"],
            "W0": np.asarray(W0, np.float32),
            "W1": np.asarray(W1, np.float32),
            "W2": np.asarray(W2, np.float32),
            "b0": np.asarray(b0, np.float32).reshape(64, 1),
            "b1": np.asarray(b1, np.float32).reshape(32, 1),
            "b2": np.asarray(b2, np.float32).reshape(4, 1),
        })

    res = run_bass_kernel_spmd(nc, in_maps, core_ids=list(range(NCORES)))

    out = np.zeros((N_NODES, 8), np.float32)
    for c, co in enumerate(cores):
        y = res.results[c]["y"].reshape(P, NTILES, 8)
        node_ids = co["node_perm"].reshape(NTILES, P)
        for t in range(NTILES):
            valid = (t * P + np.arange(P)) < NSH
            out[c * NSH + node_ids[t][valid]] = y[valid, t]
    return out


# revision 12
# speedup vs baseline: 1.1911x; 1.1689x over previous
"""Trainium2 Bass kernel: nn_EquivariantCGLayer (GNN message passing).

8 NeuronCores, tgt-sharded: core c owns nodes [c*12500,(c+1)*12500) and all
edges targeting them. Host pre-lays edges into a uniform [128 x T_pad] slot
space (degree-sorted node tiles, exact per-node windows). Device: f[src] via
chunked dma_gather of an 8-node-packed f16 table + 8-way mask-select, f[tgt]
via on-chip scalar broadcast, 48 CG raw components, two-stage tanh on ACT
(scale-grouped), per-slot messages -> DRAM msgbuf -> windowed dma_gather +
masked reduce -> node aggregates -> invariants + MLP (PE) + sigmoid gating.
"""

import math

import numpy as np

P = 128
N_NODES = 100000
NCORES = 8
NSH = N_NODES // NCORES
NTILES = math.ceil(NSH / P)          # 98
NPAD = NTILES * P                    # 12544
SQ3 = float(np.sqrt(3.0))
SQ6 = float(np.sqrt(6.0))
NI = 1024                            # descriptors per dma_gather
GCOLS = NI // P                      # 8
CHUNK = 160                          # slot columns per compute chunk
SUB = 32                             # slot columns per gather sub-chunk

GA, GD, GB, GC, GE = 0, 5, 17, 21, 36
NRAW = 48

_CACHE = {}


def _wrap_idx(logical):
    """int16 logical desc stream -> dma_gather wrapped [128, n/16] layout."""
    lg = logical.reshape(-1, 16)
    w = np.zeros((P, lg.shape[0]), np.int16)
    for g in range(8):
        w[16 * g:16 * g + 16, :] = lg.T
    return w


def _host_layout(edge_index, d, a):
    src = np.asarray(edge_index[0]).astype(np.int64)
    tgt = np.asarray(edge_index[1]).astype(np.int64)
    d = np.asarray(d, dtype=np.float32).reshape(-1)
    a = np.asarray(a, dtype=np.float32)

    cores = []
    for c in range(NCORES):
        sel = (tgt >= c * NSH) & (tgt < (c + 1) * NSH)
        cores.append(dict(src=src[sel], tgt=tgt[sel] - c * NSH,
                          a=a[sel], d=d[sel]))

    for co in cores:
        deg = np.bincount(co["tgt"], minlength=NSH)
        order = np.argsort(-deg, kind="stable")
        co["node_perm"] = np.concatenate([order, np.zeros(NPAD - NSH, np.int64)])
        co["deg_p"] = np.concatenate([deg[order], np.zeros(NPAD - NSH, np.int64)])

    W = np.ones(NTILES, np.int64)
    for co in cores:
        W = np.maximum(W, co["deg_p"].reshape(NTILES, P).max(axis=1))
    c0 = np.concatenate([[0], np.cumsum(W)]).astype(np.int64)
    T_tot = int(c0[-1])
    T_pad = math.ceil(T_tot / CHUNK) * CHUNK
    NG = T_pad // GCOLS

    # msgbuf window rows: record = 8 f16 = 16B, 16 records per 256B row
    R = np.ones(NTILES, np.int64)
    for co in cores:
        deg_pt = co["deg_p"].reshape(NTILES, P)
        for t in range(NTILES):
            st = np.arange(P) * T_pad + int(c0[t])
            en = st + np.maximum(deg_pt[t], 1) - 1
            R[t] = max(R[t], int(((en >> 4) - (st >> 4) + 1).max()))
    cumR = np.concatenate([[0], np.cumsum(R)]).astype(np.int64)
    RT = int(cumR[-1])

    for co in cores:
        a4 = np.zeros((P, T_pad, 4), np.float16)
        dpl = np.zeros((P, T_pad), np.float16)
        srclo8 = np.full((P, T_pad, 8), 64.0, np.float16)
        srcpk = np.zeros((P, T_pad), np.int16)
        eorder = np.argsort(co["tgt"], kind="stable")
        tse = co["tgt"][eorder]
        starts = np.searchsorted(tse, np.arange(NSH))
        ends = np.searchsorted(tse, np.arange(NSH) + 1)
        deg_pt = co["deg_p"].reshape(NTILES, P)
        for t in range(NTILES):
            cs = int(c0[t])
            for p in range(P):
                node = co["node_perm"][t * P + p]
                k = int(deg_pt[t, p])
                if k == 0:
                    continue
                eids = eorder[starts[node]:ends[node]]
                a4[p, cs:cs + k] = co["a"][eids]
                dpl[p, cs:cs + k] = co["d"][eids]
                sv = co["src"][eids]
                srclo8[p, cs:cs + k, :] = (sv & 7)[:, None]
                srcpk[p, cs:cs + k] = (sv >> 3).astype(np.int16)
        co["a4"] = a4.reshape(P, T_pad * 4)
        co["dpl"] = dpl
        co["srclo8"] = srclo8.reshape(P, T_pad * 8)
        co["invc"] = (
            1.0 / (co["deg_p"].reshape(NTILES, P).T.astype(np.float32) + 1e-8)
        ).astype(np.float32)  # [P, NTILES]

        # gather-1 idx stream
        logical1 = np.zeros(NG * NI, np.int16)
        for k in range(NG):
            logical1[k * NI:(k + 1) * NI] = (
                srcpk[:, k * GCOLS:(k + 1) * GCOLS].T.reshape(-1)
            )
        co["idx1"] = _wrap_idx(logical1)

        # gather-2 idx stream + masks
        logical2 = np.zeros(RT * P, np.int16)
        mask2 = np.zeros((P, RT, 16), np.float16)
        for t in range(NTILES):
            cs = int(c0[t])
            st = np.arange(P) * T_pad + cs                     # [P]
            deg = deg_pt[t]
            r0 = st >> 4
            for k in range(int(R[t])):
                col = int(cumR[t]) + k
                logical2[col * P:(col + 1) * P] = (r0 + k).astype(np.int16)
                base = (r0 + k) * 16
                rec = base[:, None] + np.arange(16)[None, :]   # [P,16]
                valid = (rec >= st[:, None]) & (rec < (st + deg)[:, None])
                mask2[:, col, :] = valid.astype(np.float16)
        rows0 = np.zeros((P, NTILES), np.int32)
        for t in range(NTILES):
            rows0[:, t] = (np.arange(P) * T_pad + int(c0[t])) >> 4
        co["idx3"] = rows0
        # replicate mask over the 8 comps -> [P, RT*128]
        co["mask2"] = np.repeat(
            mask2.reshape(P, RT * 16), 8, axis=1
        ).astype(np.float16)

    meta = dict(W=W, c0=c0, T_pad=T_pad, NG=NG, R=R, cumR=cumR, RT=RT)
    return cores, meta


def _pack_ftab(f):
    nrows = math.ceil(N_NODES / 8)
    ft = np.zeros((nrows * 8, 16), np.float16)
    ft[:N_NODES, :8] = f
    return ft.reshape(nrows, 128)


def _build_nc(meta, w1, w2):
    import concourse.bacc as bacc
    import concourse.bass as bass
    import concourse.mybir as mybir
    from concourse.tile import TileContext
    from concourse.masks import make_identity
    import contextlib

    dt = mybir.dt
    F16, F32 = dt.float16, dt.float32
    AF = mybir.ActivationFunctionType
    ALU = mybir.AluOpType
    T_pad, NG, RT = meta["T_pad"], meta["NG"], meta["RT"]
    W, c0, R, cumR = meta["W"], meta["c0"], meta["R"], meta["cumR"]
    NT = NTILES
    NROWS_FT = math.ceil(N_NODES / 8)
    MROWS = P * T_pad * 8 // 128     # msgbuf 256B rows

    w1 = [float(w1[0]), float(w1[1])]
    w2 = [float(w2[0]), float(w2[1])]

    nc = bacc.Bacc(None, target_bir_lowering=False)
    ftab = nc.dram_tensor("ftab", [NROWS_FT, 128], F16, kind="ExternalInput")
    a4_d = nc.dram_tensor("a4", [P, T_pad * 4], F16, kind="ExternalInput")
    dpl_d = nc.dram_tensor("dpl", [P, T_pad], F16, kind="ExternalInput")
    srclo8_d = nc.dram_tensor("srclo8", [P, T_pad * 8], F16, kind="ExternalInput")
    idx1_d = nc.dram_tensor("idx1", [P, NG * NI // 16], dt.int16, kind="ExternalInput")
    idx3_d = nc.dram_tensor("idx3", [P, NT], dt.int32, kind="ExternalInput")
    mask2_d = nc.dram_tensor("mask2", [P, RT * 128], F16, kind="ExternalInput")
    fall_d = nc.dram_tensor("fall", [P, NT * 8], F32, kind="ExternalInput")
    invc_d = nc.dram_tensor("invc", [P, NT], F32, kind="ExternalInput")
    W0_d = nc.dram_tensor("W0", [9, 64], F32, kind="ExternalInput")
    W1_d = nc.dram_tensor("W1", [64, 32], F32, kind="ExternalInput")
    W2_d = nc.dram_tensor("W2", [32, 4], F32, kind="ExternalInput")
    b0_d = nc.dram_tensor("b0", [64, 1], F32, kind="ExternalInput")
    b1_d = nc.dram_tensor("b1", [32, 1], F32, kind="ExternalInput")
    b2_d = nc.dram_tensor("b2", [4, 1], F32, kind="ExternalInput")
    y_d = nc.dram_tensor("y", [P, NT * 8], F32, kind="ExternalOutput")
    msg_d = nc.dram_tensor("msgbuf", [MROWS, 128], F16)

    msg_w = msg_d[:, :].rearrange("r e -> (r e)").rearrange("(p x) -> p x", p=P)

    NCH = T_pad // CHUNK
    with TileContext(nc) as tc:
        with contextlib.ExitStack() as ctx:
            pool = ctx.enter_context(tc.tile_pool(name="sbuf", bufs=2))
            rawp = ctx.enter_context(tc.tile_pool(name="rawp", bufs=1))
            pers = ctx.enter_context(tc.tile_pool(name="pers", bufs=1))
            psum = ctx.enter_context(tc.tile_pool(name="psum", bufs=1, space="PSUM"))

            fall_t = pers.tile([P, NT * 8], F32, tag="fall")
            nc.sync.dma_start(fall_t[:], fall_d[:])
            fall16 = pers.tile([P, NT * 8], F16, tag="fall16")
            nc.vector.tensor_copy(fall16[:], fall_t[:])
            agg_t = pers.tile([P, NT * 8], F32, tag="agg")

            # ---- ftwin: broadcast f[tgt] along windows ----------------------
            ftwin = pers.tile([P, T_pad * 8], F16, tag="ftwin")
            nc.vector.memset(ftwin[:], 0.0)
            for t in range(NT):
                cs, w = int(c0[t]), int(W[t])
                nc.vector.tensor_copy(
                    ftwin[:, cs * 8:(cs + w) * 8]
                    .rearrange("p (i e) -> p i e", e=8),
                    fall16[:, t * 8:(t + 1) * 8]
                    .rearrange("p (one e) -> p one e", one=1)
                    .to_broadcast([P, w, 8]),
                )

            # ---- edge pipeline ---------------------------------------------
            for ch in range(NCH):
                cs = ch * CHUNK
                nch_i = CHUNK // GCOLS
                idx1_t = pool.tile([P, nch_i * (NI // 16)], dt.int16, tag="idx1c")
                nc.sync.dma_start(
                    idx1_t[:],
                    idx1_d[:, ch * nch_i * (NI // 16):(ch + 1) * nch_i * (NI // 16)],
                )
                fs8 = pool.tile([P, CHUNK * 8], F16, tag="fs8")
                for sub in range(CHUNK // SUB):
                    fswin = pool.tile([P, SUB * P], F16, tag="fswin")
                    for g4 in range(SUB // GCOLS):
                        kk = (sub * SUB) // GCOLS + g4
                        nc.gpsimd.dma_gather(
                            out_ap=fswin[:, g4 * GCOLS * P:(g4 + 1) * GCOLS * P]
                            .rearrange("p (i e) -> p i e", e=P),
                            in_ap=ftab[:, :],
                            idxs_ap=idx1_t[:, kk * (NI // 16):(kk + 1) * (NI // 16)],
                            num_idxs=NI, num_idxs_reg=NI, elem_size=P,
                        )
                    sc0 = cs + sub * SUB
                    slo = pool.tile([P, SUB * 8], F16, tag="slo")
                    nc.sync.dma_start(slo[:], srclo8_d[:, sc0 * 8:(sc0 + SUB) * 8])
                    dst = fs8[:, sub * SUB * 8:(sub + 1) * SUB * 8]
                    for s in range(8):
                        m_s = pool.tile([P, SUB * 8], F16, tag="msk")
                        nc.vector.tensor_scalar(
                            out=m_s[:], in0=slo[:], scalar1=float(s),
                            scalar2=None, op0=ALU.is_equal,
                        )
                        win_s = (fswin[:].rearrange("p (i e) -> p i e", e=P)
                                 [:, :, 16 * s:16 * s + 8])
                        tmp = pool.tile([P, SUB * 8], F16, tag="seltmp")
                        nc.vector.tensor_tensor(
                            out=tmp[:].rearrange("p (i e) -> p i e", e=8),
                            in0=win_s,
                            in1=m_s[:].rearrange("p (i e) -> p i e", e=8),
                            op=ALU.mult,
                        )
                        if s == 0:
                            nc.vector.tensor_copy(dst, tmp[:])
                        else:
                            nc.vector.tensor_tensor(out=dst, in0=dst,
                                                    in1=tmp[:], op=ALU.add)

                a4c = pool.tile([P, CHUNK * 4], F16, tag="a4c")
                nc.sync.dma_start(a4c[:], a4_d[:, cs * 4:(cs + CHUNK) * 4])
                dc = pool.tile([P, CHUNK], F16, tag="dc")
                nc.sync.dma_start(dc[:], dpl_d[:, cs:cs + CHUNK])

                def A4(j):
                    return a4c[:, j::4]

                def FS(c):
                    return fs8[:, c::8]

                def FT(c):
                    return ftwin[:, cs * 8 + c:(cs + CHUNK) * 8:8]

                raw = rawp.tile([P, CHUNK * NRAW], F16, tag="raw")

                def RW(pl):
                    return raw[:, pl * CHUNK:(pl + 1) * CHUNK]

                def mul(o, x, y):
                    nc.vector.tensor_tensor(out=o, in0=x, in1=y, op=ALU.mult)

                def add(o, x, y):
                    nc.vector.tensor_tensor(out=o, in0=x, in1=y, op=ALU.add)

                def sub_(o, x, y):
                    nc.vector.tensor_tensor(out=o, in0=x, in1=y, op=ALU.subtract)

                tmp1 = pool.tile([P, CHUNK], F16, tag="tmp1")
                tmp2 = pool.tile([P, CHUNK], F16, tag="tmp2")
                a0 = A4(0)
                av = [A4(1), A4(2), A4(3)]
                vecs = [(FS(2), FS(3), FS(4)), (FS(5), FS(6), FS(7)),
                        (FT(2), FT(3), FT(4)), (FT(5), FT(6), FT(7))]

                for i, s in enumerate([FS(0), FS(1), FT(0), FT(1), dc[:]]):
                    mul(RW(GA + i), s, a0)
                for vi, v in enumerate(vecs):
                    for j in range(3):
                        mul(RW(GD + vi * 3 + j), v[j], a0)
                for vi, v in enumerate(vecs):
                    mul(tmp1[:], v[0], av[0])
                    mul(tmp2[:], v[1], av[1])
                    add(tmp1[:], tmp1[:], tmp2[:])
                    mul(tmp2[:], v[2], av[2])
                    add(RW(GB + vi), tmp1[:], tmp2[:])
                for i, s in enumerate([FS(0), FS(1), FT(0), FT(1), dc[:]]):
                    for j in range(3):
                        mul(RW(GC + i * 3 + j), s, av[j])
                for vi, v in enumerate(vecs):
                    pl = GE + vi * 3
                    mul(tmp1[:], v[1], av[2]); mul(tmp2[:], v[2], av[1])
                    sub_(RW(pl + 0), tmp1[:], tmp2[:])
                    mul(tmp1[:], v[2], av[0]); mul(tmp2[:], v[0], av[2])
                    sub_(RW(pl + 1), tmp1[:], tmp2[:])
                    mul(tmp1[:], v[0], av[1]); mul(tmp2[:], v[1], av[0])
                    sub_(RW(pl + 2), tmp1[:], tmp2[:])

                t1 = rawp.tile([P, CHUNK * NRAW], F16, tag="t1")
                for (pl0, npl, sc) in [(GA, 5, w1[0]), (GD, 12, w1[0] / SQ3),
                                       (GB, 4, w1[1] / SQ3), (GC, 15, w1[1] / SQ3),
                                       (GE, 12, w1[1] / SQ6)]:
                    nc.scalar.activation(
                        t1[:, pl0 * CHUNK:(pl0 + npl) * CHUNK],
                        raw[:, pl0 * CHUNK:(pl0 + npl) * CHUNK],
                        AF.Tanh, scale=float(sc),
                    )
                t2 = raw  # raw is dead after t1; reuse its buffer
                nc.scalar.activation(t2[:, :17 * CHUNK], t1[:, :17 * CHUNK],
                                     AF.Tanh, scale=w2[0])
                nc.scalar.activation(t2[:, 17 * CHUNK:], t1[:, 17 * CHUNK:],
                                     AF.Tanh, scale=w2[1])

                def T2(pl):
                    return t2[:, pl * CHUNK:(pl + 1) * CHUNK]

                mout = pool.tile([P, CHUNK * 8], F16, tag="mout")
                m0 = tmp1
                add(m0[:], T2(GA + 0), T2(GA + 1))
                for pl in [GA + 2, GA + 3, GA + 4, GB, GB + 1, GB + 2, GB + 3]:
                    add(m0[:], m0[:], T2(pl))
                nc.vector.tensor_copy(mout[:, 0::8], m0[:])
                for j in range(3):
                    mv = tmp2
                    add(mv[:], T2(GC + j), T2(GC + 3 + j))
                    for b in [GC + 6 + j, GC + 9 + j, GC + 12 + j,
                              GD + j, GD + 3 + j, GD + 6 + j, GD + 9 + j,
                              GE + j, GE + 3 + j, GE + 6 + j, GE + 9 + j]:
                        add(mv[:], mv[:], T2(b))
                    nc.vector.tensor_copy(mout[:, 1 + j::8], mv[:])
                nc.vector.tensor_copy(mout[:, 4::8], dc[:])
                nc.vector.memset(mout[:, 5::8], 0.0)
                nc.vector.memset(mout[:, 6::8], 0.0)
                nc.vector.memset(mout[:, 7::8], 0.0)
                nc.sync.dma_start(msg_w[:, cs * 8:(cs + CHUNK) * 8], mout[:])

            # ---- aggregation: per-tile indirect window DMA -----------------
            idx3_t = pers.tile([P, NT], mybir.dt.int32, tag="idx3")
            nc.sync.dma_start(idx3_t[:], idx3_d[:])
            for t in range(NT):
                r0, rt = int(cumR[t]), int(R[t])
                w2b = pool.tile([P, rt * P], F16, tag="w2b")
                nc.gpsimd.indirect_dma_start(
                    out=w2b[:],
                    out_offset=None,
                    in_=msg_d[:, :],
                    in_offset=bass.IndirectOffsetOnAxis(
                        ap=idx3_t[:, t:t + 1], axis=0
                    ),
                )
                m2 = pool.tile([P, rt * 128], F16, tag="m2")
                nc.sync.dma_start(m2[:], mask2_d[:, r0 * 128:(r0 + rt) * 128])
                nc.vector.tensor_tensor(out=w2b[:], in0=w2b[:], in1=m2[:],
                                        op=ALU.mult)
                nc.vector.tensor_reduce(
                    out=agg_t[:, t * 8:(t + 1) * 8],
                    in_=w2b[:].rearrange("p (x c) -> p c x", c=8),
                    axis=mybir.AxisListType.X, op=ALU.add,
                )

            # ---- node stage ------------------------------------------------
            invc_t = pers.tile([P, NT], F32, tag="invc")
            nc.sync.dma_start(invc_t[:], invc_d[:])
            ident = pers.tile([P, P], F32, tag="ident")
            make_identity(nc, ident[:])
            w0t = pers.tile([9, 64], F32, tag="w0")
            nc.sync.dma_start(w0t[:], W0_d[:])
            w1t = pers.tile([64, 32], F32, tag="w1")
            nc.sync.dma_start(w1t[:], W1_d[:])
            w2t = pers.tile([32, 4], F32, tag="w2")
            nc.sync.dma_start(w2t[:], W2_d[:])
            b0t = pers.tile([64, 1], F32, tag="b0")
            nc.sync.dma_start(b0t[:], b0_d[:])
            b1t = pers.tile([32, 1], F32, tag="b1")
            nc.sync.dma_start(b1t[:], b1_d[:])
            b2t = pers.tile([4, 1], F32, tag="b2")
            nc.sync.dma_start(b2t[:], b2_d[:])

            psi = pers.tile([P, NT * 9], F32, tag="psi")
            tm1 = pers.tile([P, NT], F32, tag="tm1")
            tm2 = pers.tile([P, NT], F32, tag="tm2")

            def FA(c):
                return fall_t[:, c::8]

            def AG(c):
                return agg_t[:, c::8]

            def vmul(o, x, y):
                nc.vector.tensor_tensor(out=o, in0=x, in1=y, op=ALU.mult)

            def vadd(o, x, y):
                nc.vector.tensor_tensor(out=o, in0=x, in1=y, op=ALU.add)

            # psi0..1 = f0, f1 ; psi2 = |f[2:5]| ; psi3 = |f[5:8]|
            nc.vector.tensor_copy(psi[:, 0::9], FA(0))
            nc.vector.tensor_copy(psi[:, 1::9], FA(1))
            for (k, base) in [(2, 2), (3, 5)]:
                vmul(tm1[:], FA(base), FA(base))
                vmul(tm2[:], FA(base + 1), FA(base + 1))
                vadd(tm1[:], tm1[:], tm2[:])
                vmul(tm2[:], FA(base + 2), FA(base + 2))
                vadd(tm1[:], tm1[:], tm2[:])
                nc.scalar.activation(psi[:, k::9], tm1[:], AF.Sqrt)
            # psi4,5 = m0 ; psi6,7 = |mv|
            nc.vector.tensor_copy(psi[:, 4::9], AG(0))
            nc.vector.tensor_copy(psi[:, 5::9], AG(0))
            vmul(tm1[:], AG(1), AG(1))
            vmul(tm2[:], AG(2), AG(2))
            vadd(tm1[:], tm1[:], tm2[:])
            vmul(tm2[:], AG(3), AG(3))
            vadd(tm1[:], tm1[:], tm2[:])
            nc.scalar.activation(psi[:, 6::9], tm1[:], AF.Sqrt)
            nc.vector.tensor_copy(psi[:, 7::9], psi[:, 6::9])
            # psi8 = avg_d
            vmul(psi[:, 8::9], AG(4), invc_t[:])

            gm = pers.tile([P, NT * 4], F32, tag="gm")
            NCHN = 4                      # tiles per node chunk (512 nodes)
            for q in range(NT // NCHN + (1 if NT % NCHN else 0)):
                tlo = q * NCHN
                thi = min(tlo + NCHN, NT)
                nt_q = thi - tlo
                psiT = pool.tile([9, nt_q * P], F32, tag="psiT")
                for ti in range(nt_q):
                    pst = psum.tile([9, P], F32, tag="pst", space="PSUM")
                    nc.tensor.transpose(
                        out=pst[:],
                        in_=psi[:, (tlo + ti) * 9:(tlo + ti + 1) * 9],
                        identity=ident[:],
                    )
                    nc.vector.tensor_copy(psiT[:, ti * P:(ti + 1) * P], pst[:])
                x1p = psum.tile([64, nt_q * P], F32, tag="x1p", space="PSUM")
                nc.tensor.matmul(x1p[:], lhsT=w0t[:], rhs=psiT[:], start=True,
                                 stop=True)
                x1s = pool.tile([64, nt_q * P], F32, tag="x1s")
                nc.scalar.activation(x1s[:], x1p[:], AF.Relu, bias=b0t[:, 0:1])
                x2p = psum.tile([32, nt_q * P], F32, tag="x2p", space="PSUM")
                nc.tensor.matmul(x2p[:], lhsT=w1t[:], rhs=x1s[:], start=True,
                                 stop=True)
                x2s = pool.tile([32, nt_q * P], F32, tag="x2s")
                nc.scalar.activation(x2s[:], x2p[:], AF.Relu, bias=b1t[:, 0:1])
                gp = psum.tile([4, nt_q * P], F32, tag="gp", space="PSUM")
                nc.tensor.matmul(gp[:], lhsT=w2t[:], rhs=x2s[:], start=True,
                                 stop=True)
                gs = pool.tile([4, nt_q * P], F32, tag="gs")
                nc.scalar.activation(gs[:], gp[:], AF.Sigmoid, bias=b2t[:, 0:1])
                for ti in range(nt_q):
                    gb = psum.tile([P, 4], F32, tag="gb", space="PSUM")
                    nc.tensor.transpose(
                        out=gb[:], in_=gs[:, ti * P:(ti + 1) * P],
                        identity=ident[:4, :4],
                    )
                    nc.vector.tensor_copy(
                        gm[:, (tlo + ti) * 4:(tlo + ti + 1) * 4], gb[:]
                    )

            y_t = pers.tile([P, NT * 8], F32, tag="y")
            gidx = [0, 1, 2, 2, 2, 3, 3, 3]
            aidx = [0, 0, 1, 2, 3, 1, 2, 3]
            for c in range(8):
                vmul(tm1[:], gm[:, gidx[c]::4], AG(aidx[c]))
                vadd(y_t[:, c::8], FA(c), tm1[:])
            nc.sync.dma_start(y_d[:], y_t[:])

    nc.finalize()
    return nc


def kernel(edge_index, f, d, a, w1, w2, W0, b0, W1, b1, W2, b2):
    from concourse.bass_utils import run_bass_kernel_spmd

    f = np.asarray(f, dtype=np.float32)
    w1 = np.asarray(w1, dtype=np.float32)
    w2 = np.asarray(w2, dtype=np.float32)

    cores, meta = _host_layout(edge_index, d, a)
    ftab = _pack_ftab(f)

    key = (meta["T_pad"], meta["RT"], tuple(meta["W"]), tuple(meta["R"]),
           float(w1[0]), float(w1[1]), float(w2[0]), float(w2[1]))
    if key not in _CACHE:
        _CACHE[key] = _build_nc(meta, w1, w2)
    nc = _CACHE[key]

    in_maps = []
    for c, co in enumerate(cores):
        fall = np.zeros((P, NTILES * 8), np.float32)
        node_ids = co["node_perm"].reshape(NTILES, P)
        for t in range(NTILES):
            fall[:, t * 8:(t + 1) * 8] = f[c * NSH + node_ids[t]]
        in_maps.append({
            "ftab": ftab,
            "a4": co["a4"], "dpl": co["dpl"], "srclo8": co["srclo8"],
            "idx1": co["idx1"], "idx3": co["idx3"], "mask2": co["mask2"],
            "fall": fall, "invc": co["invc"],
            "W0": np.asarray(W0, np.float32),
            "W1": np.asarray(W1, np.float32),
            "W2": np.asarray(W2, np.float32),
            "b0": np.asarray(b0, np.float32).reshape(64, 1),
            "b1": np.asarray(b1, np.float32).reshape(32, 1),
            "b2": np.asarray(b2, np.float32).reshape(4, 1),
        })

    res = run_bass_kernel_spmd(nc, in_maps, core_ids=list(range(NCORES)))

    out = np.zeros((N_NODES, 8), np.float32)
    for c, co in enumerate(cores):
        y = res.results[c]["y"].reshape(P, NTILES, 8)
        node_ids = co["node_perm"].reshape(NTILES, P)
        for t in range(NTILES):
            valid = (t * P + np.arange(P)) < NSH
            out[c * NSH + node_ids[t][valid]] = y[valid, t]
    return out


# revision 13
# speedup vs baseline: 1.2038x; 1.0107x over previous
"""Trainium2 Bass kernel: nn_EquivariantCGLayer (GNN message passing).

8 NeuronCores, tgt-sharded: core c owns nodes [c*12500,(c+1)*12500) and all
edges targeting them. Host pre-lays edges into a uniform [128 x T_pad] slot
space (degree-sorted node tiles, exact per-node windows). Device: f[src] via
chunked dma_gather of an 8-node-packed f16 table + 8-way mask-select, f[tgt]
via on-chip scalar broadcast, 48 CG raw components, two-stage tanh on ACT
(scale-grouped), per-slot messages -> DRAM msgbuf -> windowed dma_gather +
masked reduce -> node aggregates -> invariants + MLP (PE) + sigmoid gating.
"""

import math

import numpy as np

P = 128
N_NODES = 100000
NCORES = 8
NSH = N_NODES // NCORES
NTILES = math.ceil(NSH / P)          # 98
NPAD = NTILES * P                    # 12544
SQ3 = float(np.sqrt(3.0))
SQ6 = float(np.sqrt(6.0))
NI = 1024                            # descriptors per dma_gather
GCOLS = NI // P                      # 8
CHUNK = 160                          # slot columns per compute chunk
SUB = 32                             # slot columns per gather sub-chunk

GA, GD, GB, GC, GE = 0, 5, 17, 21, 36
NRAW = 48

_CACHE = {}


def _wrap_idx(logical):
    """int16 logical desc stream -> dma_gather wrapped [128, n/16] layout."""
    lg = logical.reshape(-1, 16)
    w = np.zeros((P, lg.shape[0]), np.int16)
    for g in range(8):
        w[16 * g:16 * g + 16, :] = lg.T
    return w


def _host_layout(edge_index, d, a):
    src = np.asarray(edge_index[0]).astype(np.int64)
    tgt = np.asarray(edge_index[1]).astype(np.int64)
    d = np.asarray(d, dtype=np.float32).reshape(-1)
    a = np.asarray(a, dtype=np.float32)

    cores = []
    for c in range(NCORES):
        sel = (tgt >= c * NSH) & (tgt < (c + 1) * NSH)
        cores.append(dict(src=src[sel], tgt=tgt[sel] - c * NSH,
                          a=a[sel], d=d[sel]))

    for co in cores:
        deg = np.bincount(co["tgt"], minlength=NSH)
        order = np.argsort(-deg, kind="stable")
        co["node_perm"] = np.concatenate([order, np.zeros(NPAD - NSH, np.int64)])
        co["deg_p"] = np.concatenate([deg[order], np.zeros(NPAD - NSH, np.int64)])

    W = np.ones(NTILES, np.int64)
    for co in cores:
        W = np.maximum(W, co["deg_p"].reshape(NTILES, P).max(axis=1))
    c0 = np.concatenate([[0], np.cumsum(W)]).astype(np.int64)
    T_tot = int(c0[-1])
    T_pad = math.ceil(T_tot / CHUNK) * CHUNK
    NG = T_pad // GCOLS

    # msgbuf window rows: record = 8 f16 = 16B, 16 records per 256B row
    R = np.ones(NTILES, np.int64)
    for co in cores:
        deg_pt = co["deg_p"].reshape(NTILES, P)
        for t in range(NTILES):
            st = np.arange(P) * T_pad + int(c0[t])
            en = st + np.maximum(deg_pt[t], 1) - 1
            R[t] = max(R[t], int(((en >> 4) - (st >> 4) + 1).max()))
    cumR = np.concatenate([[0], np.cumsum(R)]).astype(np.int64)
    RT = int(cumR[-1])

    for co in cores:
        a4 = np.zeros((P, T_pad, 4), np.float16)
        dpl = np.zeros((P, T_pad), np.float16)
        srclo8 = np.full((P, T_pad, 8), 64.0, np.float16)
        srcpk = np.zeros((P, T_pad), np.int16)
        eorder = np.argsort(co["tgt"], kind="stable")
        tse = co["tgt"][eorder]
        starts = np.searchsorted(tse, np.arange(NSH))
        ends = np.searchsorted(tse, np.arange(NSH) + 1)
        deg_pt = co["deg_p"].reshape(NTILES, P)
        for t in range(NTILES):
            cs = int(c0[t])
            for p in range(P):
                node = co["node_perm"][t * P + p]
                k = int(deg_pt[t, p])
                if k == 0:
                    continue
                eids = eorder[starts[node]:ends[node]]
                a4[p, cs:cs + k] = co["a"][eids]
                dpl[p, cs:cs + k] = co["d"][eids]
                sv = co["src"][eids]
                srclo8[p, cs:cs + k, :] = (sv & 7)[:, None]
                srcpk[p, cs:cs + k] = (sv >> 3).astype(np.int16)
        co["a4"] = a4.reshape(P, T_pad * 4)
        co["dpl"] = dpl
        co["srclo8"] = srclo8.reshape(P, T_pad * 8)
        co["invc"] = (
            1.0 / (co["deg_p"].reshape(NTILES, P).T.astype(np.float32) + 1e-8)
        ).astype(np.float32)  # [P, NTILES]

        # gather-1 idx stream
        logical1 = np.zeros(NG * NI, np.int16)
        for k in range(NG):
            logical1[k * NI:(k + 1) * NI] = (
                srcpk[:, k * GCOLS:(k + 1) * GCOLS].T.reshape(-1)
            )
        co["idx1"] = _wrap_idx(logical1)

        # gather-2 idx stream + masks
        logical2 = np.zeros(RT * P, np.int16)
        mask2 = np.zeros((P, RT, 16), np.float16)
        for t in range(NTILES):
            cs = int(c0[t])
            st = np.arange(P) * T_pad + cs                     # [P]
            deg = deg_pt[t]
            r0 = st >> 4
            for k in range(int(R[t])):
                col = int(cumR[t]) + k
                logical2[col * P:(col + 1) * P] = (r0 + k).astype(np.int16)
                base = (r0 + k) * 16
                rec = base[:, None] + np.arange(16)[None, :]   # [P,16]
                valid = (rec >= st[:, None]) & (rec < (st + deg)[:, None])
                mask2[:, col, :] = valid.astype(np.float16)
        rows0 = np.zeros((P, NTILES), np.int32)
        for t in range(NTILES):
            rows0[:, t] = (np.arange(P) * T_pad + int(c0[t])) >> 4
        co["idx3"] = rows0
        # replicate mask over the 8 comps -> [P, RT*128]
        co["mask2"] = np.repeat(
            mask2.reshape(P, RT * 16), 8, axis=1
        ).astype(np.float16)

    meta = dict(W=W, c0=c0, T_pad=T_pad, NG=NG, R=R, cumR=cumR, RT=RT)
    return cores, meta


def _pack_ftab(f):
    nrows = math.ceil(N_NODES / 8)
    ft = np.zeros((nrows * 8, 16), np.float16)
    ft[:N_NODES, :8] = f
    return ft.reshape(nrows, 128)


def _build_nc(meta, w1, w2):
    import concourse.bacc as bacc
    import concourse.bass as bass
    import concourse.mybir as mybir
    from concourse.tile import TileContext
    from concourse.masks import make_identity
    import contextlib

    dt = mybir.dt
    F16, F32 = dt.float16, dt.float32
    AF = mybir.ActivationFunctionType
    ALU = mybir.AluOpType
    T_pad, NG, RT = meta["T_pad"], meta["NG"], meta["RT"]
    W, c0, R, cumR = meta["W"], meta["c0"], meta["R"], meta["cumR"]
    NT = NTILES
    NROWS_FT = math.ceil(N_NODES / 8)
    MROWS = P * T_pad * 8 // 128     # msgbuf 256B rows

    w1 = [float(w1[0]), float(w1[1])]
    w2 = [float(w2[0]), float(w2[1])]

    nc = bacc.Bacc(None, target_bir_lowering=False)
    ftab = nc.dram_tensor("ftab", [NROWS_FT, 128], F16, kind="ExternalInput")
    a4_d = nc.dram_tensor("a4", [P, T_pad * 4], F16, kind="ExternalInput")
    dpl_d = nc.dram_tensor("dpl", [P, T_pad], F16, kind="ExternalInput")
    srclo8_d = nc.dram_tensor("srclo8", [P, T_pad * 8], F16, kind="ExternalInput")
    idx1_d = nc.dram_tensor("idx1", [P, NG * NI // 16], dt.int16, kind="ExternalInput")
    idx3_d = nc.dram_tensor("idx3", [P, NT], dt.int32, kind="ExternalInput")
    mask2_d = nc.dram_tensor("mask2", [P, RT * 128], F16, kind="ExternalInput")
    fall_d = nc.dram_tensor("fall", [P, NT * 8], F32, kind="ExternalInput")
    invc_d = nc.dram_tensor("invc", [P, NT], F32, kind="ExternalInput")
    W0_d = nc.dram_tensor("W0", [9, 64], F32, kind="ExternalInput")
    W1_d = nc.dram_tensor("W1", [64, 32], F32, kind="ExternalInput")
    W2_d = nc.dram_tensor("W2", [32, 4], F32, kind="ExternalInput")
    b0_d = nc.dram_tensor("b0", [64, 1], F32, kind="ExternalInput")
    b1_d = nc.dram_tensor("b1", [32, 1], F32, kind="ExternalInput")
    b2_d = nc.dram_tensor("b2", [4, 1], F32, kind="ExternalInput")
    y_d = nc.dram_tensor("y", [P, NT * 8], F32, kind="ExternalOutput")
    msg_d = nc.dram_tensor("msgbuf", [MROWS, 128], F16)

    msg_w = msg_d[:, :].rearrange("r e -> (r e)").rearrange("(p x) -> p x", p=P)

    NCH = T_pad // CHUNK
    with TileContext(nc) as tc:
        with contextlib.ExitStack() as ctx:
            pool = ctx.enter_context(tc.tile_pool(name="sbuf", bufs=2))
            gpool = ctx.enter_context(tc.tile_pool(name="gpool", bufs=4))
            rawp = ctx.enter_context(tc.tile_pool(name="rawp", bufs=1))
            pers = ctx.enter_context(tc.tile_pool(name="pers", bufs=1))
            psum = ctx.enter_context(tc.tile_pool(name="psum", bufs=1, space="PSUM"))

            fall_t = pers.tile([P, NT * 8], F32, tag="fall")
            nc.sync.dma_start(fall_t[:], fall_d[:])
            fall16 = pers.tile([P, NT * 8], F16, tag="fall16")
            nc.vector.tensor_copy(fall16[:], fall_t[:])
            agg_t = pers.tile([P, NT * 8], F32, tag="agg")

            # ---- ftwin: broadcast f[tgt] along windows ----------------------
            ftwin = pers.tile([P, T_pad * 8], F16, tag="ftwin")
            nc.vector.memset(ftwin[:], 0.0)
            for t in range(NT):
                cs, w = int(c0[t]), int(W[t])
                nc.vector.tensor_copy(
                    ftwin[:, cs * 8:(cs + w) * 8]
                    .rearrange("p (i e) -> p i e", e=8),
                    fall16[:, t * 8:(t + 1) * 8]
                    .rearrange("p (one e) -> p one e", one=1)
                    .to_broadcast([P, w, 8]),
                )

            psi = pers.tile([P, NT * 9], F32, tag="psi")
            tm1 = pers.tile([P, NT], F32, tag="tm1")
            tm2 = pers.tile([P, NT], F32, tag="tm2")

            def FA(c):
                return fall_t[:, c::8]

            def AG(c):
                return agg_t[:, c::8]

            def vmul(o, x, y):
                nc.vector.tensor_tensor(out=o, in0=x, in1=y, op=ALU.mult)

            def vadd(o, x, y):
                nc.vector.tensor_tensor(out=o, in0=x, in1=y, op=ALU.add)

            # psi0..1 = f0, f1 ; psi2 = |f[2:5]| ; psi3 = |f[5:8]|
            nc.vector.tensor_copy(psi[:, 0::9], FA(0))
            nc.vector.tensor_copy(psi[:, 1::9], FA(1))
            for (k, base) in [(2, 2), (3, 5)]:
                vmul(tm1[:], FA(base), FA(base))
                vmul(tm2[:], FA(base + 1), FA(base + 1))
                vadd(tm1[:], tm1[:], tm2[:])
                vmul(tm2[:], FA(base + 2), FA(base + 2))
                vadd(tm1[:], tm1[:], tm2[:])
                nc.scalar.activation(psi[:, k::9], tm1[:], AF.Sqrt)
            # ---- edge pipeline ---------------------------------------------
            for ch in range(NCH):
                cs = ch * CHUNK
                nch_i = CHUNK // GCOLS
                idx1_t = pool.tile([P, nch_i * (NI // 16)], dt.int16, tag="idx1c")
                nc.sync.dma_start(
                    idx1_t[:],
                    idx1_d[:, ch * nch_i * (NI // 16):(ch + 1) * nch_i * (NI // 16)],
                )
                fs8 = pool.tile([P, CHUNK * 8], F16, tag="fs8")
                for sub in range(CHUNK // SUB):
                    fswin = gpool.tile([P, SUB * P], F16, tag="fswin")
                    for g4 in range(SUB // GCOLS):
                        kk = (sub * SUB) // GCOLS + g4
                        nc.gpsimd.dma_gather(
                            out_ap=fswin[:, g4 * GCOLS * P:(g4 + 1) * GCOLS * P]
                            .rearrange("p (i e) -> p i e", e=P),
                            in_ap=ftab[:, :],
                            idxs_ap=idx1_t[:, kk * (NI // 16):(kk + 1) * (NI // 16)],
                            num_idxs=NI, num_idxs_reg=NI, elem_size=P,
                        )
                    sc0 = cs + sub * SUB
                    slo = pool.tile([P, SUB * 8], F16, tag="slo")
                    nc.sync.dma_start(slo[:], srclo8_d[:, sc0 * 8:(sc0 + SUB) * 8])
                    dst = fs8[:, sub * SUB * 8:(sub + 1) * SUB * 8]
                    for s in range(8):
                        m_s = pool.tile([P, SUB * 8], F16, tag="msk")
                        nc.vector.tensor_scalar(
                            out=m_s[:], in0=slo[:], scalar1=float(s),
                            scalar2=None, op0=ALU.is_equal,
                        )
                        win_s = (fswin[:].rearrange("p (i e) -> p i e", e=P)
                                 [:, :, 16 * s:16 * s + 8])
                        tmp = pool.tile([P, SUB * 8], F16, tag="seltmp")
                        nc.vector.tensor_tensor(
                            out=tmp[:].rearrange("p (i e) -> p i e", e=8),
                            in0=win_s,
                            in1=m_s[:].rearrange("p (i e) -> p i e", e=8),
                            op=ALU.mult,
                        )
                        if s == 0:
                            nc.vector.tensor_copy(dst, tmp[:])
                        else:
                            nc.vector.tensor_tensor(out=dst, in0=dst,
                                                    in1=tmp[:], op=ALU.add)

                a4c = pool.tile([P, CHUNK * 4], F16, tag="a4c")
                nc.sync.dma_start(a4c[:], a4_d[:, cs * 4:(cs + CHUNK) * 4])
                dc = pool.tile([P, CHUNK], F16, tag="dc")
                nc.sync.dma_start(dc[:], dpl_d[:, cs:cs + CHUNK])

                def A4(j):
                    return a4c[:, j::4]

                def FS(c):
                    return fs8[:, c::8]

                def FT(c):
                    return ftwin[:, cs * 8 + c:(cs + CHUNK) * 8:8]

                raw = rawp.tile([P, CHUNK * NRAW], F16, tag="raw")

                def RW(pl):
                    return raw[:, pl * CHUNK:(pl + 1) * CHUNK]

                def mul(o, x, y):
                    nc.vector.tensor_tensor(out=o, in0=x, in1=y, op=ALU.mult)

                def add(o, x, y):
                    nc.vector.tensor_tensor(out=o, in0=x, in1=y, op=ALU.add)

                def sub_(o, x, y):
                    nc.vector.tensor_tensor(out=o, in0=x, in1=y, op=ALU.subtract)

                tmp1 = pool.tile([P, CHUNK], F16, tag="tmp1")
                tmp2 = pool.tile([P, CHUNK], F16, tag="tmp2")
                a0 = A4(0)
                av = [A4(1), A4(2), A4(3)]
                vecs = [(FS(2), FS(3), FS(4)), (FS(5), FS(6), FS(7)),
                        (FT(2), FT(3), FT(4)), (FT(5), FT(6), FT(7))]

                for i, s in enumerate([FS(0), FS(1), FT(0), FT(1), dc[:]]):
                    mul(RW(GA + i), s, a0)
                for vi, v in enumerate(vecs):
                    for j in range(3):
                        mul(RW(GD + vi * 3 + j), v[j], a0)
                for vi, v in enumerate(vecs):
                    mul(tmp1[:], v[0], av[0])
                    mul(tmp2[:], v[1], av[1])
                    add(tmp1[:], tmp1[:], tmp2[:])
                    mul(tmp2[:], v[2], av[2])
                    add(RW(GB + vi), tmp1[:], tmp2[:])
                for i, s in enumerate([FS(0), FS(1), FT(0), FT(1), dc[:]]):
                    for j in range(3):
                        mul(RW(GC + i * 3 + j), s, av[j])
                for vi, v in enumerate(vecs):
                    pl = GE + vi * 3
                    mul(tmp1[:], v[1], av[2]); mul(tmp2[:], v[2], av[1])
                    sub_(RW(pl + 0), tmp1[:], tmp2[:])
                    mul(tmp1[:], v[2], av[0]); mul(tmp2[:], v[0], av[2])
                    sub_(RW(pl + 1), tmp1[:], tmp2[:])
                    mul(tmp1[:], v[0], av[1]); mul(tmp2[:], v[1], av[0])
                    sub_(RW(pl + 2), tmp1[:], tmp2[:])

                t1 = rawp.tile([P, CHUNK * NRAW], F16, tag="t1")
                for (pl0, npl, sc) in [(GA, 5, w1[0]), (GD, 12, w1[0] / SQ3),
                                       (GB, 4, w1[1] / SQ3), (GC, 15, w1[1] / SQ3),
                                       (GE, 12, w1[1] / SQ6)]:
                    nc.scalar.activation(
                        t1[:, pl0 * CHUNK:(pl0 + npl) * CHUNK],
                        raw[:, pl0 * CHUNK:(pl0 + npl) * CHUNK],
                        AF.Tanh, scale=float(sc),
                    )
                t2 = raw  # raw is dead after t1; reuse its buffer
                nc.scalar.activation(t2[:, :17 * CHUNK], t1[:, :17 * CHUNK],
                                     AF.Tanh, scale=w2[0])
                nc.scalar.activation(t2[:, 17 * CHUNK:], t1[:, 17 * CHUNK:],
                                     AF.Tanh, scale=w2[1])

                def T2(pl):
                    return t2[:, pl * CHUNK:(pl + 1) * CHUNK]

                mout = pool.tile([P, CHUNK * 8], F16, tag="mout")
                m0 = tmp1
                add(m0[:], T2(GA + 0), T2(GA + 1))
                for pl in [GA + 2, GA + 3, GA + 4, GB, GB + 1, GB + 2, GB + 3]:
                    add(m0[:], m0[:], T2(pl))
                nc.vector.tensor_copy(mout[:, 0::8], m0[:])
                for j in range(3):
                    mv = tmp2
                    add(mv[:], T2(GC + j), T2(GC + 3 + j))
                    for b in [GC + 6 + j, GC + 9 + j, GC + 12 + j,
                              GD + j, GD + 3 + j, GD + 6 + j, GD + 9 + j,
                              GE + j, GE + 3 + j, GE + 6 + j, GE + 9 + j]:
                        add(mv[:], mv[:], T2(b))
                    nc.vector.tensor_copy(mout[:, 1 + j::8], mv[:])
                nc.vector.tensor_copy(mout[:, 4::8], dc[:])
                nc.vector.memset(mout[:, 5::8], 0.0)
                nc.vector.memset(mout[:, 6::8], 0.0)
                nc.vector.memset(mout[:, 7::8], 0.0)
                nc.sync.dma_start(msg_w[:, cs * 8:(cs + CHUNK) * 8], mout[:])

            # ---- aggregation: per-tile indirect window DMA -----------------
            idx3_t = pers.tile([P, NT], mybir.dt.int32, tag="idx3")
            nc.sync.dma_start(idx3_t[:], idx3_d[:])
            for t in range(NT):
                r0, rt = int(cumR[t]), int(R[t])
                w2b = pool.tile([P, rt * P], F16, tag="w2b")
                nc.gpsimd.indirect_dma_start(
                    out=w2b[:],
                    out_offset=None,
                    in_=msg_d[:, :],
                    in_offset=bass.IndirectOffsetOnAxis(
                        ap=idx3_t[:, t:t + 1], axis=0
                    ),
                )
                m2 = pool.tile([P, rt * 128], F16, tag="m2")
                nc.sync.dma_start(m2[:], mask2_d[:, r0 * 128:(r0 + rt) * 128])
                nc.vector.tensor_tensor(out=w2b[:], in0=w2b[:], in1=m2[:],
                                        op=ALU.mult)
                nc.vector.tensor_reduce(
                    out=agg_t[:, t * 8:(t + 1) * 8],
                    in_=w2b[:].rearrange("p (x c) -> p c x", c=8),
                    axis=mybir.AxisListType.X, op=ALU.add,
                )

            # ---- node stage ------------------------------------------------
            invc_t = pers.tile([P, NT], F32, tag="invc")
            nc.sync.dma_start(invc_t[:], invc_d[:])
            ident = pers.tile([P, P], F32, tag="ident")
            make_identity(nc, ident[:])
            w0t = pers.tile([9, 64], F32, tag="w0")
            nc.sync.dma_start(w0t[:], W0_d[:])
            w1t = pers.tile([64, 32], F32, tag="w1")
            nc.sync.dma_start(w1t[:], W1_d[:])
            w2t = pers.tile([32, 4], F32, tag="w2")
            nc.sync.dma_start(w2t[:], W2_d[:])
            b0t = pers.tile([64, 1], F32, tag="b0")
            nc.sync.dma_start(b0t[:], b0_d[:])
            b1t = pers.tile([32, 1], F32, tag="b1")
            nc.sync.dma_start(b1t[:], b1_d[:])
            b2t = pers.tile([4, 1], F32, tag="b2")
            nc.sync.dma_start(b2t[:], b2_d[:])

            # psi4,5 = m0 ; psi6,7 = |mv|
            nc.vector.tensor_copy(psi[:, 4::9], AG(0))
            nc.vector.tensor_copy(psi[:, 5::9], AG(0))
            vmul(tm1[:], AG(1), AG(1))
            vmul(tm2[:], AG(2), AG(2))
            vadd(tm1[:], tm1[:], tm2[:])
            vmul(tm2[:], AG(3), AG(3))
            vadd(tm1[:], tm1[:], tm2[:])
            nc.scalar.activation(psi[:, 6::9], tm1[:], AF.Sqrt)
            nc.vector.tensor_copy(psi[:, 7::9], psi[:, 6::9])
            # psi8 = avg_d
            vmul(psi[:, 8::9], AG(4), invc_t[:])

            gm = pers.tile([P, NT * 4], F32, tag="gm")
            NCHN = 4                      # tiles per node chunk (512 nodes)
            for q in range(NT // NCHN + (1 if NT % NCHN else 0)):
                tlo = q * NCHN
                thi = min(tlo + NCHN, NT)
                nt_q = thi - tlo
                psiT = pool.tile([9, nt_q * P], F32, tag="psiT")
                for ti in range(nt_q):
                    pst = psum.tile([9, P], F32, tag="pst", space="PSUM")
                    nc.tensor.transpose(
                        out=pst[:],
                        in_=psi[:, (tlo + ti) * 9:(tlo + ti + 1) * 9],
                        identity=ident[:],
                    )
                    nc.vector.tensor_copy(psiT[:, ti * P:(ti + 1) * P], pst[:])
                x1p = psum.tile([64, nt_q * P], F32, tag="x1p", space="PSUM")
                nc.tensor.matmul(x1p[:], lhsT=w0t[:], rhs=psiT[:], start=True,
                                 stop=True)
                x1s = pool.tile([64, nt_q * P], F32, tag="x1s")
                nc.scalar.activation(x1s[:], x1p[:], AF.Relu, bias=b0t[:, 0:1])
                x2p = psum.tile([32, nt_q * P], F32, tag="x2p", space="PSUM")
                nc.tensor.matmul(x2p[:], lhsT=w1t[:], rhs=x1s[:], start=True,
                                 stop=True)
                x2s = pool.tile([32, nt_q * P], F32, tag="x2s")
                nc.scalar.activation(x2s[:], x2p[:], AF.Relu, bias=b1t[:, 0:1])
                gp = psum.tile([4, nt_q * P], F32, tag="gp", space="PSUM")
                nc.tensor.matmul(gp[:], lhsT=w2t[:], rhs=x2s[:], start=True,
                                 stop=True)
                gs = pool.tile([4, nt_q * P], F32, tag="gs")
                nc.scalar.activation(gs[:], gp[:], AF.Sigmoid, bias=b2t[:, 0:1])
                for ti in range(nt_q):
                    gb = psum.tile([P, 4], F32, tag="gb", space="PSUM")
                    nc.tensor.transpose(
                        out=gb[:], in_=gs[:, ti * P:(ti + 1) * P],
                        identity=ident[:4, :4],
                    )
                    nc.vector.tensor_copy(
                        gm[:, (tlo + ti) * 4:(tlo + ti + 1) * 4], gb[:]
                    )

            y_t = pers.tile([P, NT * 8], F32, tag="y")
            gidx = [0, 1, 2, 2, 2, 3, 3, 3]
            aidx = [0, 0, 1, 2, 3, 1, 2, 3]
            for c in range(8):
                vmul(tm1[:], gm[:, gidx[c]::4], AG(aidx[c]))
                vadd(y_t[:, c::8], FA(c), tm1[:])
            nc.sync.dma_start(y_d[:], y_t[:])

    nc.finalize()
    return nc


def kernel(edge_index, f, d, a, w1, w2, W0, b0, W1, b1, W2, b2):
    from concourse.bass_utils import run_bass_kernel_spmd

    f = np.asarray(f, dtype=np.float32)
    w1 = np.asarray(w1, dtype=np.float32)
    w2 = np.asarray(w2, dtype=np.float32)

    cores, meta = _host_layout(edge_index, d, a)
    ftab = _pack_ftab(f)

    key = (meta["T_pad"], meta["RT"], tuple(meta["W"]), tuple(meta["R"]),
           float(w1[0]), float(w1[1]), float(w2[0]), float(w2[1]))
    if key not in _CACHE:
        _CACHE[key] = _build_nc(meta, w1, w2)
    nc = _CACHE[key]

    in_maps = []
    for c, co in enumerate(cores):
        fall = np.zeros((P, NTILES * 8), np.float32)
        node_ids = co["node_perm"].reshape(NTILES, P)
        for t in range(NTILES):
            fall[:, t * 8:(t + 1) * 8] = f[c * NSH + node_ids[t]]
        in_maps.append({
            "ftab": ftab,
            "a4": co["a4"], "dpl": co["dpl"], "srclo8": co["srclo8"],
            "idx1": co["idx1"], "idx3": co["idx3"], "mask2": co["mask2"],
            "fall": fall, "invc": co["invc"],
            "W0": np.asarray(W0, np.float32),
            "W1": np.asarray(W1, np.float32),
            "W2": np.asarray(W2, np.float32),
            "b0": np.asarray(b0, np.float32).reshape(64, 1),
            "b1": np.asarray(b1, np.float32).reshape(32, 1),
            "b2": np.asarray(b2, np.float32).reshape(4, 1),
        })

    res = run_bass_kernel_spmd(nc, in_maps, core_ids=list(range(NCORES)))

    out = np.zeros((N_NODES, 8), np.float32)
    for c, co in enumerate(cores):
        y = res.results[c]["y"].reshape(P, NTILES, 8)
        node_ids = co["node_perm"].reshape(NTILES, P)
        for t in range(NTILES):
            valid = (t * P + np.arange(P)) < NSH
            out[c * NSH + node_ids[t][valid]] = y[valid, t]
    return out


# revision 14
# speedup vs baseline: 1.2043x; 1.0004x over previous
"""Trainium2 Bass kernel: nn_EquivariantCGLayer (GNN message passing).

8 NeuronCores, tgt-sharded: core c owns nodes [c*12500,(c+1)*12500) and all
edges targeting them. Host pre-lays edges into a uniform [128 x T_pad] slot
space (degree-sorted node tiles, exact per-node windows). Device: f[src] via
chunked dma_gather of an 8-node-packed f16 table + 8-way mask-select, f[tgt]
via on-chip scalar broadcast, 48 CG raw components, two-stage tanh on ACT
(scale-grouped), per-slot messages -> DRAM msgbuf -> windowed dma_gather +
masked reduce -> node aggregates -> invariants + MLP (PE) + sigmoid gating.
"""

import math

import numpy as np

P = 128
N_NODES = 100000
NCORES = 8
NSH = N_NODES // NCORES
NTILES = math.ceil(NSH / P)          # 98
NPAD = NTILES * P                    # 12544
SQ3 = float(np.sqrt(3.0))
SQ6 = float(np.sqrt(6.0))
NI = 1024                            # descriptors per dma_gather
GCOLS = NI // P                      # 8
CHUNK = 160                          # slot columns per compute chunk
SUB = 32                             # slot columns per gather sub-chunk

GA, GD, GB, GC, GE = 0, 5, 17, 21, 36
NRAW = 48

_CACHE = {}


def _wrap_idx(logical):
    """int16 logical desc stream -> dma_gather wrapped [128, n/16] layout."""
    lg = logical.reshape(-1, 16)
    w = np.zeros((P, lg.shape[0]), np.int16)
    for g in range(8):
        w[16 * g:16 * g + 16, :] = lg.T
    return w


def _host_layout(edge_index, d, a):
    src = np.asarray(edge_index[0]).astype(np.int64)
    tgt = np.asarray(edge_index[1]).astype(np.int64)
    d = np.asarray(d, dtype=np.float32).reshape(-1)
    a = np.asarray(a, dtype=np.float32)

    cores = []
    for c in range(NCORES):
        sel = (tgt >= c * NSH) & (tgt < (c + 1) * NSH)
        cores.append(dict(src=src[sel], tgt=tgt[sel] - c * NSH,
                          a=a[sel], d=d[sel]))

    for co in cores:
        deg = np.bincount(co["tgt"], minlength=NSH)
        order = np.argsort(-deg, kind="stable")
        co["node_perm"] = np.concatenate([order, np.zeros(NPAD - NSH, np.int64)])
        co["deg_p"] = np.concatenate([deg[order], np.zeros(NPAD - NSH, np.int64)])

    W = np.ones(NTILES, np.int64)
    for co in cores:
        W = np.maximum(W, co["deg_p"].reshape(NTILES, P).max(axis=1))
    c0 = np.concatenate([[0], np.cumsum(W)]).astype(np.int64)
    T_tot = int(c0[-1])
    T_pad = math.ceil(T_tot / CHUNK) * CHUNK
    NG = T_pad // GCOLS

    # msgbuf window rows: record = 8 f16 = 16B, 16 records per 256B row
    R = np.ones(NTILES, np.int64)
    for co in cores:
        deg_pt = co["deg_p"].reshape(NTILES, P)
        for t in range(NTILES):
            st = np.arange(P) * T_pad + int(c0[t])
            en = st + np.maximum(deg_pt[t], 1) - 1
            R[t] = max(R[t], int(((en >> 4) - (st >> 4) + 1).max()))
    cumR = np.concatenate([[0], np.cumsum(R)]).astype(np.int64)
    RT = int(cumR[-1])

    for co in cores:
        a4 = np.zeros((P, T_pad, 4), np.float16)
        dpl = np.zeros((P, T_pad), np.float16)
        srclo8 = np.full((P, T_pad, 8), 64.0, np.float16)
        srcpk = np.zeros((P, T_pad), np.int16)
        eorder = np.argsort(co["tgt"], kind="stable")
        tse = co["tgt"][eorder]
        starts = np.searchsorted(tse, np.arange(NSH))
        ends = np.searchsorted(tse, np.arange(NSH) + 1)
        deg_pt = co["deg_p"].reshape(NTILES, P)
        for t in range(NTILES):
            cs = int(c0[t])
            for p in range(P):
                node = co["node_perm"][t * P + p]
                k = int(deg_pt[t, p])
                if k == 0:
                    continue
                eids = eorder[starts[node]:ends[node]]
                a4[p, cs:cs + k] = co["a"][eids]
                dpl[p, cs:cs + k] = co["d"][eids]
                sv = co["src"][eids]
                srclo8[p, cs:cs + k, :] = (sv & 7)[:, None]
                srcpk[p, cs:cs + k] = (sv >> 3).astype(np.int16)
        co["a4"] = a4.reshape(P, T_pad * 4)
        co["dpl"] = dpl
        co["srclo8"] = srclo8.reshape(P, T_pad * 8)
        co["invc"] = (
            1.0 / (co["deg_p"].reshape(NTILES, P).T.astype(np.float32) + 1e-8)
        ).astype(np.float32)  # [P, NTILES]

        # gather-1 idx stream
        logical1 = np.zeros(NG * NI, np.int16)
        for k in range(NG):
            logical1[k * NI:(k + 1) * NI] = (
                srcpk[:, k * GCOLS:(k + 1) * GCOLS].T.reshape(-1)
            )
        co["idx1"] = _wrap_idx(logical1)

        # gather-2 idx stream + masks
        logical2 = np.zeros(RT * P, np.int16)
        mask2 = np.zeros((P, RT, 16), np.float16)
        for t in range(NTILES):
            cs = int(c0[t])
            st = np.arange(P) * T_pad + cs                     # [P]
            deg = deg_pt[t]
            r0 = st >> 4
            for k in range(int(R[t])):
                col = int(cumR[t]) + k
                logical2[col * P:(col + 1) * P] = (r0 + k).astype(np.int16)
                base = (r0 + k) * 16
                rec = base[:, None] + np.arange(16)[None, :]   # [P,16]
                valid = (rec >= st[:, None]) & (rec < (st + deg)[:, None])
                mask2[:, col, :] = valid.astype(np.float16)
        rows0 = np.zeros((P, NTILES), np.int32)
        for t in range(NTILES):
            rows0[:, t] = (np.arange(P) * T_pad + int(c0[t])) >> 4
        co["idx3"] = rows0
        # replicate mask over the 8 comps -> [P, RT*128]
        co["mask2"] = np.repeat(
            mask2.reshape(P, RT * 16), 8, axis=1
        ).astype(np.float16)

    meta = dict(W=W, c0=c0, T_pad=T_pad, NG=NG, R=R, cumR=cumR, RT=RT)
    return cores, meta


def _pack_ftab(f):
    nrows = math.ceil(N_NODES / 8)
    ft = np.zeros((nrows * 8, 16), np.float16)
    ft[:N_NODES, :8] = f
    return ft.reshape(nrows, 128)


def _build_nc(meta, w1, w2):
    import concourse.bacc as bacc
    import concourse.bass as bass
    import concourse.mybir as mybir
    from concourse.tile import TileContext
    from concourse.masks import make_identity
    import contextlib

    dt = mybir.dt
    F16, F32 = dt.float16, dt.float32
    AF = mybir.ActivationFunctionType
    ALU = mybir.AluOpType
    T_pad, NG, RT = meta["T_pad"], meta["NG"], meta["RT"]
    W, c0, R, cumR = meta["W"], meta["c0"], meta["R"], meta["cumR"]
    NT = NTILES
    NROWS_FT = math.ceil(N_NODES / 8)
    MROWS = P * T_pad * 8 // 128     # msgbuf 256B rows

    w1 = [float(w1[0]), float(w1[1])]
    w2 = [float(w2[0]), float(w2[1])]

    nc = bacc.Bacc(None, target_bir_lowering=False)
    ftab = nc.dram_tensor("ftab", [NROWS_FT, 128], F16, kind="ExternalInput")
    a4_d = nc.dram_tensor("a4", [P, T_pad * 4], F16, kind="ExternalInput")
    dpl_d = nc.dram_tensor("dpl", [P, T_pad], F16, kind="ExternalInput")
    srclo8_d = nc.dram_tensor("srclo8", [P, T_pad * 8], F16, kind="ExternalInput")
    idx1_d = nc.dram_tensor("idx1", [P, NG * NI // 16], dt.int16, kind="ExternalInput")
    idx3_d = nc.dram_tensor("idx3", [P, NT], dt.int32, kind="ExternalInput")
    mask2_d = nc.dram_tensor("mask2", [P, RT * 128], F16, kind="ExternalInput")
    fall_d = nc.dram_tensor("fall", [P, NT * 8], F32, kind="ExternalInput")
    invc_d = nc.dram_tensor("invc", [P, NT], F32, kind="ExternalInput")
    W0_d = nc.dram_tensor("W0", [9, 64], F32, kind="ExternalInput")
    W1_d = nc.dram_tensor("W1", [64, 32], F32, kind="ExternalInput")
    W2_d = nc.dram_tensor("W2", [32, 4], F32, kind="ExternalInput")
    b0_d = nc.dram_tensor("b0", [64, 1], F32, kind="ExternalInput")
    b1_d = nc.dram_tensor("b1", [32, 1], F32, kind="ExternalInput")
    b2_d = nc.dram_tensor("b2", [4, 1], F32, kind="ExternalInput")
    y_d = nc.dram_tensor("y", [P, NT * 8], F32, kind="ExternalOutput")
    msg_d = nc.dram_tensor("msgbuf", [MROWS, 128], F16)

    msg_w = msg_d[:, :].rearrange("r e -> (r e)").rearrange("(p x) -> p x", p=P)

    NCH = T_pad // CHUNK
    with TileContext(nc) as tc:
        with contextlib.ExitStack() as ctx:
            pool = ctx.enter_context(tc.tile_pool(name="sbuf", bufs=2))
            gpool = ctx.enter_context(tc.tile_pool(name="gpool", bufs=4))
            rawp = ctx.enter_context(tc.tile_pool(name="rawp", bufs=2))
            pers = ctx.enter_context(tc.tile_pool(name="pers", bufs=1))
            psum = ctx.enter_context(tc.tile_pool(name="psum", bufs=1, space="PSUM"))

            fall_t = pers.tile([P, NT * 8], F32, tag="fall")
            nc.sync.dma_start(fall_t[:], fall_d[:])
            fall16 = pers.tile([P, NT * 8], F16, tag="fall16")
            nc.vector.tensor_copy(fall16[:], fall_t[:])
            agg_t = pers.tile([P, NT * 8], F32, tag="agg")

            # ---- ftwin: broadcast f[tgt] along windows ----------------------
            ftwin = pers.tile([P, T_pad * 8], F16, tag="ftwin")
            nc.vector.memset(ftwin[:], 0.0)
            for t in range(NT):
                cs, w = int(c0[t]), int(W[t])
                nc.vector.tensor_copy(
                    ftwin[:, cs * 8:(cs + w) * 8]
                    .rearrange("p (i e) -> p i e", e=8),
                    fall16[:, t * 8:(t + 1) * 8]
                    .rearrange("p (one e) -> p one e", one=1)
                    .to_broadcast([P, w, 8]),
                )

            psi = pers.tile([P, NT * 9], F32, tag="psi")
            tm1 = pers.tile([P, NT], F32, tag="tm1")
            tm2 = pers.tile([P, NT], F32, tag="tm2")

            def FA(c):
                return fall_t[:, c::8]

            def AG(c):
                return agg_t[:, c::8]

            def vmul(o, x, y):
                nc.vector.tensor_tensor(out=o, in0=x, in1=y, op=ALU.mult)

            def vadd(o, x, y):
                nc.vector.tensor_tensor(out=o, in0=x, in1=y, op=ALU.add)

            # psi0..1 = f0, f1 ; psi2 = |f[2:5]| ; psi3 = |f[5:8]|
            nc.vector.tensor_copy(psi[:, 0::9], FA(0))
            nc.vector.tensor_copy(psi[:, 1::9], FA(1))
            for (k, base) in [(2, 2), (3, 5)]:
                vmul(tm1[:], FA(base), FA(base))
                vmul(tm2[:], FA(base + 1), FA(base + 1))
                vadd(tm1[:], tm1[:], tm2[:])
                vmul(tm2[:], FA(base + 2), FA(base + 2))
                vadd(tm1[:], tm1[:], tm2[:])
                nc.scalar.activation(psi[:, k::9], tm1[:], AF.Sqrt)
            # ---- edge pipeline ---------------------------------------------
            for ch in range(NCH):
                cs = ch * CHUNK
                nch_i = CHUNK // GCOLS
                idx1_t = pool.tile([P, nch_i * (NI // 16)], dt.int16, tag="idx1c")
                nc.sync.dma_start(
                    idx1_t[:],
                    idx1_d[:, ch * nch_i * (NI // 16):(ch + 1) * nch_i * (NI // 16)],
                )
                fs8 = pool.tile([P, CHUNK * 8], F16, tag="fs8")
                for sub in range(CHUNK // SUB):
                    fswin = gpool.tile([P, SUB * P], F16, tag="fswin")
                    for g4 in range(SUB // GCOLS):
                        kk = (sub * SUB) // GCOLS + g4
                        nc.gpsimd.dma_gather(
                            out_ap=fswin[:, g4 * GCOLS * P:(g4 + 1) * GCOLS * P]
                            .rearrange("p (i e) -> p i e", e=P),
                            in_ap=ftab[:, :],
                            idxs_ap=idx1_t[:, kk * (NI // 16):(kk + 1) * (NI // 16)],
                            num_idxs=NI, num_idxs_reg=NI, elem_size=P,
                        )
                    sc0 = cs + sub * SUB
                    slo = pool.tile([P, SUB * 8], F16, tag="slo")
                    nc.sync.dma_start(slo[:], srclo8_d[:, sc0 * 8:(sc0 + SUB) * 8])
                    dst = fs8[:, sub * SUB * 8:(sub + 1) * SUB * 8]
                    for s in range(8):
                        m_s = pool.tile([P, SUB * 8], F16, tag="msk")
                        nc.vector.tensor_scalar(
                            out=m_s[:], in0=slo[:], scalar1=float(s),
                            scalar2=None, op0=ALU.is_equal,
                        )
                        win_s = (fswin[:].rearrange("p (i e) -> p i e", e=P)
                                 [:, :, 16 * s:16 * s + 8])
                        tmp = pool.tile([P, SUB * 8], F16, tag="seltmp")
                        nc.vector.tensor_tensor(
                            out=tmp[:].rearrange("p (i e) -> p i e", e=8),
                            in0=win_s,
                            in1=m_s[:].rearrange("p (i e) -> p i e", e=8),
                            op=ALU.mult,
                        )
                        if s == 0:
                            nc.vector.tensor_copy(dst, tmp[:])
                        else:
                            nc.vector.tensor_tensor(out=dst, in0=dst,
                                                    in1=tmp[:], op=ALU.add)

                a4c = pool.tile([P, CHUNK * 4], F16, tag="a4c")
                nc.sync.dma_start(a4c[:], a4_d[:, cs * 4:(cs + CHUNK) * 4])
                dc = pool.tile([P, CHUNK], F16, tag="dc")
                nc.sync.dma_start(dc[:], dpl_d[:, cs:cs + CHUNK])

                def A4(j):
                    return a4c[:, j::4]

                def FS(c):
                    return fs8[:, c::8]

                def FT(c):
                    return ftwin[:, cs * 8 + c:(cs + CHUNK) * 8:8]

                raw = rawp.tile([P, CHUNK * NRAW], F16, tag="raw")

                def RW(pl):
                    return raw[:, pl * CHUNK:(pl + 1) * CHUNK]

                def mul(o, x, y):
                    nc.vector.tensor_tensor(out=o, in0=x, in1=y, op=ALU.mult)

                def add(o, x, y):
                    nc.vector.tensor_tensor(out=o, in0=x, in1=y, op=ALU.add)

                def sub_(o, x, y):
                    nc.vector.tensor_tensor(out=o, in0=x, in1=y, op=ALU.subtract)

                tmp1 = pool.tile([P, CHUNK], F16, tag="tmp1")
                tmp2 = pool.tile([P, CHUNK], F16, tag="tmp2")
                a0 = A4(0)
                av = [A4(1), A4(2), A4(3)]
                vecs = [(FS(2), FS(3), FS(4)), (FS(5), FS(6), FS(7)),
                        (FT(2), FT(3), FT(4)), (FT(5), FT(6), FT(7))]

                for i, s in enumerate([FS(0), FS(1), FT(0), FT(1), dc[:]]):
                    mul(RW(GA + i), s, a0)
                for vi, v in enumerate(vecs):
                    for j in range(3):
                        mul(RW(GD + vi * 3 + j), v[j], a0)
                for vi, v in enumerate(vecs):
                    mul(tmp1[:], v[0], av[0])
                    mul(tmp2[:], v[1], av[1])
                    add(tmp1[:], tmp1[:], tmp2[:])
                    mul(tmp2[:], v[2], av[2])
                    add(RW(GB + vi), tmp1[:], tmp2[:])
                for i, s in enumerate([FS(0), FS(1), FT(0), FT(1), dc[:]]):
                    for j in range(3):
                        mul(RW(GC + i * 3 + j), s, av[j])
                for vi, v in enumerate(vecs):
                    pl = GE + vi * 3
                    mul(tmp1[:], v[1], av[2]); mul(tmp2[:], v[2], av[1])
                    sub_(RW(pl + 0), tmp1[:], tmp2[:])
                    mul(tmp1[:], v[2], av[0]); mul(tmp2[:], v[0], av[2])
                    sub_(RW(pl + 1), tmp1[:], tmp2[:])
                    mul(tmp1[:], v[0], av[1]); mul(tmp2[:], v[1], av[0])
                    sub_(RW(pl + 2), tmp1[:], tmp2[:])

                t1 = rawp.tile([P, CHUNK * NRAW], F16, tag="t1")
                for (pl0, npl, sc) in [(GA, 5, w1[0]), (GD, 12, w1[0] / SQ3),
                                       (GB, 4, w1[1] / SQ3), (GC, 15, w1[1] / SQ3),
                                       (GE, 12, w1[1] / SQ6)]:
                    nc.scalar.activation(
                        t1[:, pl0 * CHUNK:(pl0 + npl) * CHUNK],
                        raw[:, pl0 * CHUNK:(pl0 + npl) * CHUNK],
                        AF.Tanh, scale=float(sc),
                    )
                t2 = raw  # raw is dead after t1; reuse its buffer
                nc.scalar.activation(t2[:, :17 * CHUNK], t1[:, :17 * CHUNK],
                                     AF.Tanh, scale=w2[0])
                nc.scalar.activation(t2[:, 17 * CHUNK:], t1[:, 17 * CHUNK:],
                                     AF.Tanh, scale=w2[1])

                def T2(pl):
                    return t2[:, pl * CHUNK:(pl + 1) * CHUNK]

                mout = pool.tile([P, CHUNK * 8], F16, tag="mout")
                m0 = tmp1
                add(m0[:], T2(GA + 0), T2(GA + 1))
                for pl in [GA + 2, GA + 3, GA + 4, GB, GB + 1, GB + 2, GB + 3]:
                    add(m0[:], m0[:], T2(pl))
                nc.vector.tensor_copy(mout[:, 0::8], m0[:])
                for j in range(3):
                    mv = tmp2
                    add(mv[:], T2(GC + j), T2(GC + 3 + j))
                    for b in [GC + 6 + j, GC + 9 + j, GC + 12 + j,
                              GD + j, GD + 3 + j, GD + 6 + j, GD + 9 + j,
                              GE + j, GE + 3 + j, GE + 6 + j, GE + 9 + j]:
                        add(mv[:], mv[:], T2(b))
                    nc.vector.tensor_copy(mout[:, 1 + j::8], mv[:])
                nc.vector.tensor_copy(mout[:, 4::8], dc[:])
                nc.vector.memset(mout[:, 5::8], 0.0)
                nc.vector.memset(mout[:, 6::8], 0.0)
                nc.vector.memset(mout[:, 7::8], 0.0)
                nc.sync.dma_start(msg_w[:, cs * 8:(cs + CHUNK) * 8], mout[:])

            # ---- aggregation: per-tile indirect window DMA -----------------
            idx3_t = pers.tile([P, NT], mybir.dt.int32, tag="idx3")
            nc.sync.dma_start(idx3_t[:], idx3_d[:])
            for t in range(NT):
                r0, rt = int(cumR[t]), int(R[t])
                w2b = pool.tile([P, rt * P], F16, tag="w2b")
                nc.gpsimd.indirect_dma_start(
                    out=w2b[:],
                    out_offset=None,
                    in_=msg_d[:, :],
                    in_offset=bass.IndirectOffsetOnAxis(
                        ap=idx3_t[:, t:t + 1], axis=0
                    ),
                )
                m2 = pool.tile([P, rt * 128], F16, tag="m2")
                nc.sync.dma_start(m2[:], mask2_d[:, r0 * 128:(r0 + rt) * 128])
                nc.vector.tensor_tensor(out=w2b[:], in0=w2b[:], in1=m2[:],
                                        op=ALU.mult)
                nc.vector.tensor_reduce(
                    out=agg_t[:, t * 8:(t + 1) * 8],
                    in_=w2b[:].rearrange("p (x c) -> p c x", c=8),
                    axis=mybir.AxisListType.X, op=ALU.add,
                )

            # ---- node stage ------------------------------------------------
            invc_t = pers.tile([P, NT], F32, tag="invc")
            nc.sync.dma_start(invc_t[:], invc_d[:])
            ident = pers.tile([P, P], F32, tag="ident")
            make_identity(nc, ident[:])
            w0t = pers.tile([9, 64], F32, tag="w0")
            nc.sync.dma_start(w0t[:], W0_d[:])
            w1t = pers.tile([64, 32], F32, tag="w1")
            nc.sync.dma_start(w1t[:], W1_d[:])
            w2t = pers.tile([32, 4], F32, tag="w2")
            nc.sync.dma_start(w2t[:], W2_d[:])
            b0t = pers.tile([64, 1], F32, tag="b0")
            nc.sync.dma_start(b0t[:], b0_d[:])
            b1t = pers.tile([32, 1], F32, tag="b1")
            nc.sync.dma_start(b1t[:], b1_d[:])
            b2t = pers.tile([4, 1], F32, tag="b2")
            nc.sync.dma_start(b2t[:], b2_d[:])

            # psi4,5 = m0 ; psi6,7 = |mv|
            nc.vector.tensor_copy(psi[:, 4::9], AG(0))
            nc.vector.tensor_copy(psi[:, 5::9], AG(0))
            vmul(tm1[:], AG(1), AG(1))
            vmul(tm2[:], AG(2), AG(2))
            vadd(tm1[:], tm1[:], tm2[:])
            vmul(tm2[:], AG(3), AG(3))
            vadd(tm1[:], tm1[:], tm2[:])
            nc.scalar.activation(psi[:, 6::9], tm1[:], AF.Sqrt)
            nc.vector.tensor_copy(psi[:, 7::9], psi[:, 6::9])
            # psi8 = avg_d
            vmul(psi[:, 8::9], AG(4), invc_t[:])

            gm = pers.tile([P, NT * 4], F32, tag="gm")
            NCHN = 4                      # tiles per node chunk (512 nodes)
            for q in range(NT // NCHN + (1 if NT % NCHN else 0)):
                tlo = q * NCHN
                thi = min(tlo + NCHN, NT)
                nt_q = thi - tlo
                psiT = pool.tile([9, nt_q * P], F32, tag="psiT")
                for ti in range(nt_q):
                    pst = psum.tile([9, P], F32, tag="pst", space="PSUM")
                    nc.tensor.transpose(
                        out=pst[:],
                        in_=psi[:, (tlo + ti) * 9:(tlo + ti + 1) * 9],
                        identity=ident[:],
                    )
                    nc.vector.tensor_copy(psiT[:, ti * P:(ti + 1) * P], pst[:])
                x1p = psum.tile([64, nt_q * P], F32, tag="x1p", space="PSUM")
                nc.tensor.matmul(x1p[:], lhsT=w0t[:], rhs=psiT[:], start=True,
                                 stop=True)
                x1s = pool.tile([64, nt_q * P], F32, tag="x1s")
                nc.scalar.activation(x1s[:], x1p[:], AF.Relu, bias=b0t[:, 0:1])
                x2p = psum.tile([32, nt_q * P], F32, tag="x2p", space="PSUM")
                nc.tensor.matmul(x2p[:], lhsT=w1t[:], rhs=x1s[:], start=True,
                                 stop=True)
                x2s = pool.tile([32, nt_q * P], F32, tag="x2s")
                nc.scalar.activation(x2s[:], x2p[:], AF.Relu, bias=b1t[:, 0:1])
                gp = psum.tile([4, nt_q * P], F32, tag="gp", space="PSUM")
                nc.tensor.matmul(gp[:], lhsT=w2t[:], rhs=x2s[:], start=True,
                                 stop=True)
                gs = pool.tile([4, nt_q * P], F32, tag="gs")
                nc.scalar.activation(gs[:], gp[:], AF.Sigmoid, bias=b2t[:, 0:1])
                for ti in range(nt_q):
                    gb = psum.tile([P, 4], F32, tag="gb", space="PSUM")
                    nc.tensor.transpose(
                        out=gb[:], in_=gs[:, ti * P:(ti + 1) * P],
                        identity=ident[:4, :4],
                    )
                    nc.vector.tensor_copy(
                        gm[:, (tlo + ti) * 4:(tlo + ti + 1) * 4], gb[:]
                    )

            y_t = pers.tile([P, NT * 8], F32, tag="y")
            gidx = [0, 1, 2, 2, 2, 3, 3, 3]
            aidx = [0, 0, 1, 2, 3, 1, 2, 3]
            for c in range(8):
                vmul(tm1[:], gm[:, gidx[c]::4], AG(aidx[c]))
                vadd(y_t[:, c::8], FA(c), tm1[:])
            nc.sync.dma_start(y_d[:], y_t[:])

    nc.finalize()
    return nc


def kernel(edge_index, f, d, a, w1, w2, W0, b0, W1, b1, W2, b2):
    from concourse.bass_utils import run_bass_kernel_spmd

    f = np.asarray(f, dtype=np.float32)
    w1 = np.asarray(w1, dtype=np.float32)
    w2 = np.asarray(w2, dtype=np.float32)

    cores, meta = _host_layout(edge_index, d, a)
    ftab = _pack_ftab(f)

    key = (meta["T_pad"], meta["RT"], tuple(meta["W"]), tuple(meta["R"]),
           float(w1[0]), float(w1[1]), float(w2[0]), float(w2[1]))
    if key not in _CACHE:
        _CACHE[key] = _build_nc(meta, w1, w2)
    nc = _CACHE[key]

    in_maps = []
    for c, co in enumerate(cores):
        fall = np.zeros((P, NTILES * 8), np.float32)
        node_ids = co["node_perm"].reshape(NTILES, P)
        for t in range(NTILES):
            fall[:, t * 8:(t + 1) * 8] = f[c * NSH + node_ids[t]]
        in_maps.append({
            "ftab": ftab,
            "a4": co["a4"], "dpl": co["dpl"], "srclo8": co["srclo8"],
            "idx1": co["idx1"], "idx3": co["idx3"], "mask2": co["mask2"],
            "fall": fall, "invc": co["invc"],
            "W0": np.asarray(W0, np.float32),
            "W1": np.asarray(W1, np.float32),
            "W2": np.asarray(W2, np.float32),
            "b0": np.asarray(b0, np.float32).reshape(64, 1),
            "b1": np.asarray(b1, np.float32).reshape(32, 1),
            "b2": np.asarray(b2, np.float32).reshape(4, 1),
        })

    res = run_bass_kernel_spmd(nc, in_maps, core_ids=list(range(NCORES)))

    out = np.zeros((N_NODES, 8), np.float32)
    for c, co in enumerate(cores):
        y = res.results[c]["y"].reshape(P, NTILES, 8)
        node_ids = co["node_perm"].reshape(NTILES, P)
        for t in range(NTILES):
            valid = (t * P + np.arange(P)) < NSH
            out[c * NSH + node_ids[t][valid]] = y[valid, t]
    return out
